# revision 8
# baseline (speedup 1.0000x reference)
"""Trainium2 Bass kernel for LlamaAttention (B=1, S=2048, H=4096, 32 heads).

Sharding: tensor-parallel over heads. 8 cores x 4 heads. Each core:
  - QKV projections in dual-fp8 (e4m3 hi + same-scale residual lo) using
    DoubleRow matmuls (256-deep contraction, 0.5 cyc/out-col): psum +=
    Hhi*Whi + Hhi*Wlo + Hlo*Whi. ~bf16 accuracy at 75% of bf16 cycles.
  - RoPE on Q^T/K^T (rotate-half = partition swap via SBUF DMA); descales
    folded into cos/sin tables and eviction copies.
  - causal attention in transposed layout (keys on partitions), bf16
    scores / exp / PV; per-block skip of fully-masked blocks; softmax
    without max subtraction; column sums via ones-matmul.
  - attention output quantized to dual-fp8 in SBUF; o_proj in DoubleRow
    dual-fp8; partial po written bf16. Host sums 8 partials + transposes.
"""

import os
import sys

if "/opt/trn_rl_repo" not in sys.path:
    sys.path.insert(0, "/opt/trn_rl_repo")

import numpy as np
import ml_dtypes

from concourse import bacc, mybir, tile
from concourse import bass
from concourse.bass_utils import run_bass_kernel_spmd

F32 = mybir.dt.float32
F32R = mybir.dt.float32r
BF16 = mybir.dt.bfloat16
F8 = mybir.dt.float8e4
EXPF = mybir.ActivationFunctionType.Exp
COPYF = mybir.ActivationFunctionType.Copy
DR = mybir.MatmulPerfMode.DoubleRow

N_CORES = 8
S = 2048
H = 4096
N_HEADS = 32
D = 128                      # head dim
HPC = N_HEADS // N_CORES     # heads per core = 4
HC = HPC * D                 # per-core hidden slice = 512
CH = 512                     # seq chunk width
NCH = S // CH                # 4 chunks
KT256 = H // 256             # 16 DoubleRow contraction super-tiles
SJT = S // 128               # 16 seq j-tiles
ROPE_BASE = 10000.0
NEG = -1.0e9

SH = 4.0                     # fp8 scale for hidden states
SWQ = 1024.0                 # fp8 scale for Wq*(1/sqrt d)
SWK = 64.0                   # fp8 scale for Wk / Wv
SWO = 64.0                   # fp8 scale for Wo
SAT = 4.0                    # fp8 scale for attention output
DSC = 1.0 / (SH * SWK)       # shared K/V descale (folded into tables/evicts)
QX = SWK / SWQ               # extra Q descale applied in raw copy (1/16)

F8NP = ml_dtypes.float8_e4m3
F8MAX = 240.0
BFNP = ml_dtypes.bfloat16

last_exec_time_ns = None


def _r(x):
    return np.ascontiguousarray(x, dtype=np.float32)


def _build(causal: bool, oproj_dual: bool = True):
    nc = bacc.Bacc("TRN2", target_bir_lowering=False, debug=False,
                   num_devices=N_CORES)
    ht8h = nc.dram_tensor("ht8h", [KT256, 128, NCH * 1024], F8,
                          kind="ExternalInput")
    ht8l = nc.dram_tensor("ht8l", [KT256, 128, NCH * 1024], F8,
                          kind="ExternalInput")
    wq8h = nc.dram_tensor("wq8h", [KT256, 128, 1024], F8, kind="ExternalInput")
    wq8l = nc.dram_tensor("wq8l", [KT256, 128, 1024], F8, kind="ExternalInput")
    wk8h = nc.dram_tensor("wk8h", [KT256, 128, 1024], F8, kind="ExternalInput")
    wk8l = nc.dram_tensor("wk8l", [KT256, 128, 1024], F8, kind="ExternalInput")
    wv8h = nc.dram_tensor("wv8h", [KT256, 128, 1024], F8, kind="ExternalInput")
    wv8l = nc.dram_tensor("wv8l", [KT256, 128, 1024], F8, kind="ExternalInput")
    if oproj_dual:
        wo8h = nc.dram_tensor("wo8h", [2, 128, 2 * H], F8,
                              kind="ExternalInput")
        wo8l = nc.dram_tensor("wo8l", [2, 128, 2 * H], F8,
                              kind="ExternalInput")
    else:
        wo = nc.dram_tensor("wo", [HC, H], BF16, kind="ExternalInput")
    cosb = nc.dram_tensor("cosb", [D, S], BF16, kind="ExternalInput")
    sinb = nc.dram_tensor("sinb", [D, S], BF16, kind="ExternalInput")
    if causal:
        mband = nc.dram_tensor("mband", [128, 896], BF16,
                               kind="ExternalInput")
    else:
        maskT = nc.dram_tensor("maskT", [S, S], F32, kind="ExternalInput")
    po = nc.dram_tensor("po", [H, S], BF16, kind="ExternalOutput")

    def mm(out, lhsT, rhs, start, stop):
        nc.tensor.matmul(out, lhsT, rhs, start=start, stop=stop)

    def mm8(out, lhsT, rhs, start, stop):
        nc.tensor.matmul(out, lhsT, rhs, start=start, stop=stop,
                         perf_mode=DR)

    from contextlib import ExitStack
    with tile.TileContext(nc) as tc:
        at_pool_cm = tc.tile_pool(name="at", bufs=2)
        at_pool = at_pool_cm.__enter__()
        AT_DT = F8 if oproj_dual else BF16
        ATh = at_pool.tile([128, HPC, S], AT_DT, tag="ath", name="ATh")
        if oproj_dual:
            ATl = at_pool.tile([128, HPC, S], F8, tag="atl", name="ATl")

        es_res = ExitStack()
        kt_pool = es_res.enter_context(tc.tile_pool(name="kt", bufs=HPC))
        v_pool = es_res.enter_context(tc.tile_pool(name="v", bufs=SJT))
        wqk_pool = es_res.enter_context(tc.tile_pool(name="wqk", bufs=4 * KT256))
        KT = [kt_pool.tile([128, S], BF16, tag="kt", name=f"KT{i}")
              for i in range(HPC)]
        V = [v_pool.tile([128, HC], BF16, tag="v", name=f"V{i}")
             for i in range(SJT)]
        # resident Q/K weights (dual fp8, DoubleRow pair layout)
        WQh = [wqk_pool.tile([128, 2, HC], F8, tag="w", name=f"WQh{t}")
               for t in range(KT256)]
        WQl = [wqk_pool.tile([128, 2, HC], F8, tag="w", name=f"WQl{t}")
               for t in range(KT256)]
        WKh = [wqk_pool.tile([128, 2, HC], F8, tag="w", name=f"WKh{t}")
               for t in range(KT256)]
        WKl = [wqk_pool.tile([128, 2, HC], F8, tag="w", name=f"WKl{t}")
               for t in range(KT256)]
        for t in range(KT256):
            nc.sync.dma_start(out=WQh[t][:], in_=wq8h[t])
            nc.sync.dma_start(out=WQl[t][:], in_=wq8l[t])
            nc.sync.dma_start(out=WKh[t][:], in_=wk8h[t])
            nc.sync.dma_start(out=WKl[t][:], in_=wk8l[t])

        with tc.tile_pool(name="qtc", bufs=6) as qtp, \
             tc.tile_pool(name="ht", bufs=2 * KT256 + 2) as htp, \
             tc.tile_pool(name="wvs", bufs=6) as wvp, \
             tc.tile_pool(name="cs", bufs=4) as csp, \
             tc.tile_pool(name="rope", bufs=2) as rp, \
             tc.tile_pool(name="aconst", bufs=1) as cpool, \
             tc.tile_pool(name="aes", bufs=3) as esp, \
             tc.tile_pool(name="am", bufs=1 if causal else 3) as mpool, \
             tc.tile_pool(name="ar", bufs=2) as rpool, \
             tc.tile_pool(name="mainps", bufs=6, space="PSUM") as psp:
            ones_col32 = cpool.tile([128, 1], F32, tag="oc32")
            nc.vector.memset(ones_col32[:], 1.0)
            ones_col = cpool.tile([128, 1], BF16, tag="oc")
            nc.vector.tensor_copy(ones_col[:], ones_col32[:])
            ones_row32 = cpool.tile([1, 128], F32, tag="or32")
            nc.vector.memset(ones_row32[:], SAT if oproj_dual else 1.0)
            ones_row = cpool.tile([1, 128], F32R, tag="or")
            nc.vector.tensor_copy(ones_row[:], ones_row32[:])
            if causal:
                mb = cpool.tile([128, 896], BF16, tag="mb", name="mb")
                nc.sync.dma_start(out=mb[:], in_=mband[:, :])

            def rope_evict(ps, dst_ap, cosc, sinc, qscale):
                # dst = psum*cos + shift(psum)*sin_signed  (tables hold
                # the fp8 descale; Q applies extra 1/16 in the raw copy)
                raw = rp.tile([128, CH], F32, tag="raw", name="raw")
                if qscale:
                    nc.scalar.activation(raw[:], ps[:], COPYF, scale=QX)
                else:
                    nc.scalar.copy(out=raw[:], in_=ps[:])
                shf = rp.tile([128, CH], F32, tag="shf", name="shf")
                nc.gpsimd.dma_start(out=shf[0:64, :], in_=raw[64:128, :])
                nc.gpsimd.dma_start(out=shf[64:128, :], in_=raw[0:64, :])
                tmp = rp.tile([128, CH], F32, tag="tmp", name="tmp")
                nc.vector.tensor_mul(tmp[:], shf[:], sinc[:])
                nc.vector.tensor_mul(dst_ap, raw[:], cosc[:])
                nc.vector.tensor_add(dst_ap, dst_ap, tmp[:])

            for c in range(NCH):
                cosc = csp.tile([128, CH], BF16, tag="cs", name="cosc")
                sinc = csp.tile([128, CH], BF16, tag="cs", name="sinc")
                nc.sync.dma_start(out=cosc[:], in_=cosb[:, bass.ts(c, CH)])
                nc.sync.dma_start(out=sinc[:], in_=sinb[:, bass.ts(c, CH)])
                hth = []
                htl = []
                for t in range(KT256):
                    hh = htp.tile([128, 2, CH], F8, tag="ht", name="hh")
                    nc.sync.dma_start(
                        out=hh[:], in_=ht8h[t][:, bass.ts(c, 1024)])
                    hl = htp.tile([128, 2, CH], F8, tag="ht", name="hl")
                    nc.sync.dma_start(
                        out=hl[:], in_=ht8l[t][:, bass.ts(c, 1024)])
                    hth.append(hh)
                    htl.append(hl)
                # ---- Q pass ----
                QTc = [qtp.tile([128, CH], BF16, tag="qtc", name=f"QTc{i}")
                       for i in range(HPC)]
                qps = [psp.tile([128, CH], F32, tag="ps", name=f"qps{i}")
                       for i in range(HPC)]
                for t in range(KT256):
                    st, sp = (t == 0), (t == KT256 - 1)
                    for d in range(HPC):
                        w_hi = WQh[t][:, :, bass.ts(d, 128)]
                        w_lo = WQl[t][:, :, bass.ts(d, 128)]
                        mm8(qps[d][:], w_hi, hth[t][:], st, False)
                        mm8(qps[d][:], w_lo, hth[t][:], False, False)
                        mm8(qps[d][:], w_hi, htl[t][:], False, sp)
                for d in range(HPC):
                    rope_evict(qps[d], QTc[d][:], cosc, sinc, True)
                # ---- K pass ----
                kps = [psp.tile([128, CH], F32, tag="ps", name=f"kps{i}")
                       for i in range(HPC)]
                for t in range(KT256):
                    st, sp = (t == 0), (t == KT256 - 1)
                    for d in range(HPC):
                        w_hi = WKh[t][:, :, bass.ts(d, 128)]
                        w_lo = WKl[t][:, :, bass.ts(d, 128)]
                        mm8(kps[d][:], w_hi, hth[t][:], st, False)
                        mm8(kps[d][:], w_lo, hth[t][:], False, False)
                        mm8(kps[d][:], w_hi, htl[t][:], False, sp)
                for d in range(HPC):
                    rope_evict(kps[d], KT[d][:, bass.ts(c, CH)], cosc, sinc,
                               False)
                # ---- V pass (wv streamed) ----
                vps = [psp.tile([128, HC], F32, tag="ps", name=f"vps{i}")
                       for i in range(HPC)]
                for t in range(KT256):
                    wvh = wvp.tile([128, 2, HC], F8, tag="wv", name="wvh")
                    nc.gpsimd.dma_start(out=wvh[:], in_=wv8h[t])
                    wvl = wvp.tile([128, 2, HC], F8, tag="wv", name="wvl")
                    nc.gpsimd.dma_start(out=wvl[:], in_=wv8l[t])
                    st, sp = (t == 0), (t == KT256 - 1)
                    for jl in range(4):
                        h_hi = hth[t][:, :, bass.ts(jl, 128)]
                        h_lo = htl[t][:, :, bass.ts(jl, 128)]
                        mm8(vps[jl][:], h_hi, wvh[:], st, False)
                        mm8(vps[jl][:], h_hi, wvl[:], False, False)
                        mm8(vps[jl][:], h_lo, wvh[:], False, sp)
                for jl in range(4):
                    nc.scalar.activation(V[4 * c + jl][:], vps[jl][:],
                                         COPYF, scale=DSC)

                # ---- attention for i-chunk c (K/V chunks <= c) ----
                ic = c
                jmax = 4 * ic + 4 if causal else SJT
                for h in range(HPC):
                    sum_ps = psp.tile([1, CH], F32, tag="sum", bufs=1,
                                      name="sum_ps")
                    o_ps = psp.tile([128, CH], F32, tag="o", bufs=1,
                                    name="o_ps")
                    # software-pipelined: emit scores(j+1) before sum/PV(j)
                    es_list = []
                    for j in range(jmax):
                        s_ps = psp.tile([128, CH], F32, tag="ps",
                                        name="s_ps")
                        mm(s_ps[:], KT[h][:, bass.ts(j, 128)], QTc[h][:],
                           True, True)
                        if causal:
                            if j >= 4 * ic:
                                tl = j - 4 * ic
                                off = 384 - tl * 128
                                nc.vector.tensor_add(
                                    s_ps[:], s_ps[:],
                                    mb[:, off:off + CH])
                        else:
                            mt = mpool.tile([128, CH], F32, tag="mt",
                                            name="mt")
                            nc.sync.dma_start(
                                out=mt[:],
                                in_=maskT[bass.ts(j, 128), bass.ts(ic, CH)])
                            nc.vector.tensor_add(s_ps[:], s_ps[:], mt[:])
                        es_t = esp.tile([128, CH], BF16, tag="es",
                                        name="es_t")
                        nc.scalar.activation(es_t[:], s_ps[:], EXPF)
                        es_list.append((j, es_t))
                        if len(es_list) > 1:
                            pj, pes = es_list.pop(0)
                            stq, spq = (pj == 0), (pj == jmax - 1)
                            mm(sum_ps[:], ones_col[:], pes[:], stq, spq)
                            mm(o_ps[:], V[pj][:, bass.ts(h, 128)], pes[:],
                               stq, spq)
                    pj, pes = es_list.pop(0)
                    stq, spq = (pj == 0), (pj == jmax - 1)
                    mm(sum_ps[:], ones_col[:], pes[:], stq, spq)
                    mm(o_ps[:], V[pj][:, bass.ts(h, 128)], pes[:], stq, spq)

                    rsum = rpool.tile([1, CH], F32R, tag="rs", name="rsum")
                    with nc.allow_low_precision(reason="f32r softmax recip"):
                        nc.vector.reciprocal(rsum[:], sum_ps[:])
                    b_ps = psp.tile([128, CH], F32, tag="ps", name="b_ps")
                    mm(b_ps[:], ones_row[:], rsum[:], True, True)
                    rb = rpool.tile([128, CH], BF16, tag="rb", name="rb")
                    nc.scalar.copy(out=rb[:], in_=b_ps[:])
                    if oproj_dual:
                        t4 = rpool.tile([128, CH], F32, tag="t4", name="t4")
                        nc.vector.tensor_mul(t4[:], o_ps[:], rb[:])
                        hi_ap = ATh[:, h, bass.ts(ic, CH)]
                        nc.vector.tensor_copy(hi_ap, t4[:])
                        nc.vector.tensor_sub(
                            ATl[:, h, bass.ts(ic, CH)], t4[:], hi_ap)
                    else:
                        nc.vector.tensor_mul(
                            ATh[:, h, bass.ts(ic, CH)], o_ps[:], rb[:])
        es_res.close()   # free weights/KT/V/ht SBUF before o_proj

        # ---------- o_proj  po = wo^T @ attnT ----------
        with tc.tile_pool(name="o_wo", bufs=4) as wop, \
             tc.tile_pool(name="o_out", bufs=4) as outp, \
             tc.tile_pool(name="o_ps", bufs=4, space="PSUM") as psp:
            if oproj_dual:
                WOh = [wop.tile([128, 2, H], F8, tag="wo", name=f"WOh{t}")
                       for t in range(2)]
                WOl = [wop.tile([128, 2, H], F8, tag="wo", name=f"WOl{t}")
                       for t in range(2)]
                for t in range(2):
                    nc.sync.dma_start(out=WOh[t][:], in_=wo8h[t])
                    nc.sync.dma_start(out=WOl[t][:], in_=wo8l[t])
                for ic in range(NCH):
                    for n in range(H // 128):
                        pps = psp.tile([128, CH], F32, tag="ps", name="pps")
                        for t in range(2):
                            at_hi = ATh[:, 2 * t:2 * t + 2, bass.ts(ic, CH)]
                            at_lo = ATl[:, 2 * t:2 * t + 2, bass.ts(ic, CH)]
                            w_hi = WOh[t][:, :, bass.ts(n, 128)]
                            w_lo = WOl[t][:, :, bass.ts(n, 128)]
                            mm8(pps[:], w_hi, at_hi, t == 0, False)
                            mm8(pps[:], w_lo, at_hi, False, False)
                            mm8(pps[:], w_hi, at_lo, False, t == 1)
                        ot = outp.tile([128, CH], BF16, tag="ot", name="ot")
                        nc.scalar.activation(ot[:], pps[:], COPYF,
                                             scale=1.0 / (SAT * SWO))
                        nc.gpsimd.dma_start(
                            out=po[bass.ts(n, 128), bass.ts(ic, CH)],
                            in_=ot[:])
            else:
                WO = [wop.tile([128, H], BF16, tag="wo", name=f"WO{i}")
                      for i in range(HPC)]
                for kl in range(HPC):
                    nc.sync.dma_start(out=WO[kl][:],
                                      in_=wo[bass.ts(kl, 128), :])
                for ic in range(NCH):
                    for n in range(H // 128):
                        pps = psp.tile([128, CH], F32, tag="ps", name="pps")
                        for kl in range(HPC):
                            mm(pps[:], WO[kl][:, bass.ts(n, 128)],
                               ATh[:, kl, bass.ts(ic, CH)],
                               kl == 0, kl == HPC - 1)
                        ot = outp.tile([128, CH], BF16, tag="ot", name="ot")
                        nc.scalar.copy(out=ot[:], in_=pps[:])
                        nc.gpsimd.dma_start(
                            out=po[bass.ts(n, 128), bass.ts(ic, CH)],
                            in_=ot[:])
        at_pool_cm.__exit__(None, None, None)
    nc.compile()
    return nc


_CACHE = {}


def _get_nc(causal, oproj_dual):
    key = (causal, oproj_dual)
    if key not in _CACHE:
        _CACHE[key] = _build(causal, oproj_dual)
    return _CACHE[key]


def _c8s(x):
    """clip+cast a pre-scaled f32 array to e4m3 (returns fp8, still scaled)."""
    return np.clip(x, -F8MAX, F8MAX).astype(F8NP)


def _dual8(x, s):
    hi = _c8s(np.asarray(x, np.float32) * s)
    lo = _c8s(np.asarray(x, np.float32) * s - hi.astype(np.float32))
    return hi, lo


def _pair_rows(a):
    """[R, C] fp8 -> [R/256, 128, 2*C]: DoubleRow pair layout."""
    R, C = a.shape
    return np.ascontiguousarray(
        a.reshape(R // 256, 2, 128, C).transpose(0, 2, 1, 3)
        .reshape(R // 256, 128, 2 * C))


def kernel(hidden_states, attention_mask, position_ids, Wq, Wk, Wv, Wo):
    global last_exec_time_ns
    B, S_, H_ = hidden_states.shape
    assert (B, S_, H_) == (1, S, H)
    hs = np.asarray(hidden_states, dtype=np.float32)
    mask = np.asarray(attention_mask, dtype=np.float32)[0, 0]
    pos = np.asarray(position_ids)[0].astype(np.float64)

    # causal-mask fast path check
    iu = np.triu_indices(S, k=1)
    il = np.tril_indices(S, k=0)
    causal = bool(np.all(mask[il] == 0.0) and np.all(mask[iu] <= -1e30))

    hT = np.asarray(hs[0]).T               # [H, S]
    scale = 1.0 / np.sqrt(D)

    inv_freq = 1.0 / (ROPE_BASE ** (np.arange(0, D, 2, dtype=np.float64) / D))
    ang = pos[None, :] * np.concatenate([inv_freq, inv_freq])[:, None]  # [D,S]
    cosb = np.ascontiguousarray((np.cos(ang) * DSC).astype(BFNP))
    sgn = np.ones((D, 1)); sgn[: D // 2] = -1.0
    sinb = np.ascontiguousarray((np.sin(ang) * sgn * DSC).astype(BFNP))

    # hidden states: dual fp8, chunk-interleaved pair layout [16,128,4096]
    h_hi, h_lo = _dual8(hT, SH)

    def _ht_pack(a):
        # [H, S] -> [16, 128, NCH*1024]; col index c*1024 + j*512 + col
        b = a.reshape(KT256, 2, 128, NCH, CH)       # [t, j, p, c, col]
        return np.ascontiguousarray(
            b.transpose(0, 2, 3, 1, 4).reshape(KT256, 128, NCH * 1024))

    ht8h = _ht_pack(h_hi)
    ht8l = _ht_pack(h_lo)

    wq_hi, wq_lo = _dual8(np.asarray(Wq, np.float64) * scale, SWQ)
    wk_hi, wk_lo = _dual8(np.asarray(Wk, np.float32), SWK)
    wv_hi, wv_lo = _dual8(np.asarray(Wv, np.float32), SWK)
    oproj_dual = bool(int(os.environ.get("BASS_OPROJ_DUAL", "1")))
    if oproj_dual:
        wo_hi, wo_lo = _dual8(np.asarray(Wo, np.float32), SWO)
    else:
        wo_bf = np.asarray(Wo, np.float32).astype(BFNP)

    if causal:
        # band mask tile [128, 896]: mb[r, y] = NEG iff r > y - 384
        rr = np.arange(128)[:, None]
        yy = np.arange(896)[None, :]
        mband = np.where(rr > yy - 384, NEG, 0.0).astype(BFNP)
    else:
        maskT = _r(mask.T)

    nc = _get_nc(causal, oproj_dual)
    in_maps = []
    for c in range(N_CORES):
        sl = slice(c * HC, (c + 1) * HC)
        m = {
            "ht8h": ht8h, "ht8l": ht8l,
            "wq8h": _pair_rows(wq_hi[:, sl]),
            "wq8l": _pair_rows(wq_lo[:, sl]),
            "wk8h": _pair_rows(wk_hi[:, sl]),
            "wk8l": _pair_rows(wk_lo[:, sl]),
            "wv8h": _pair_rows(wv_hi[:, sl]),
            "wv8l": _pair_rows(wv_lo[:, sl]),
            "cosb": cosb,
            "sinb": sinb,
        }
        if oproj_dual:
            m["wo8h"] = _pair_rows(wo_hi[sl, :])
            m["wo8l"] = _pair_rows(wo_lo[sl, :])
        else:
            m["wo"] = np.ascontiguousarray(wo_bf[sl, :])
        if causal:
            m["mband"] = mband
        else:
            m["maskT"] = maskT
        in_maps.append(m)

    trace = bool(int(os.environ.get("BASS_KERNEL_TRACE", "0")))
    kw = {}
    if trace:
        kw["trace"] = True
        kw["tmpdir"] = os.environ.get("BASS_KERNEL_TRACE_DIR") or None
    res = run_bass_kernel_spmd(nc, in_maps, list(range(N_CORES)), **kw)
    last_exec_time_ns = res.exec_time_ns

    acc = np.zeros((H, S), dtype=np.float32)
    for c in range(N_CORES):
        acc += res.results[c]["po"].astype(np.float32)
    out = acc.T.reshape(1, S, H)
    return out


# revision 9
# speedup vs baseline: 1.2244x; 1.2244x over previous
"""Trainium2 Bass kernel for LlamaAttention (B=1, S=2048, H=4096, 32 heads).

Sharding: tensor-parallel over heads. 8 cores x 4 heads. Each core:
  - QKV projections in bf16 (1 cyc/out-col on the PE at 2.4 GHz; fp8
    DoubleRow measured at the same out-col rate, so bf16 wins once
    accuracy needs >1 fp8 pass). Wq/Wk resident in SBUF, Wv streamed.
  - RoPE on Q^T/K^T (rotate-half = partition swap via SBUF DMA).
  - causal attention in transposed layout (keys on partitions), bf16
    scores / exp / PV, software-pipelined so exp (Act engine) overlaps
    the next block's matmuls; per-block skip of fully-masked blocks;
    softmax without max subtraction; column sums via ones-matmul.
  - attention output kept in SBUF (bf16); o_proj bf16; partial po
    written bf16. Host sums the 8 partials and transposes. No
    collectives.
"""

import os
import sys

if "/opt/trn_rl_repo" not in sys.path:
    sys.path.insert(0, "/opt/trn_rl_repo")

import numpy as np
import ml_dtypes

from concourse import bacc, mybir, tile
from concourse import bass
from concourse.bass_utils import run_bass_kernel_spmd

F32 = mybir.dt.float32
F32R = mybir.dt.float32r
BF16 = mybir.dt.bfloat16
EXPF = mybir.ActivationFunctionType.Exp

N_CORES = 8
S = 2048
H = 4096
N_HEADS = 32
D = 128                      # head dim
HPC = N_HEADS // N_CORES     # heads per core = 4
HC = HPC * D                 # per-core hidden slice = 512
CH = 512                     # seq chunk width
NCH = S // CH                # 4 chunks
KT_TILES = H // 128          # 32 contraction tiles for projections
SJT = S // 128               # 16 seq j-tiles
ROPE_BASE = 10000.0
NEG = -1.0e9

BFNP = ml_dtypes.bfloat16

last_exec_time_ns = None


def _r(x):
    return np.ascontiguousarray(x, dtype=np.float32)


def _b(x):
    return np.ascontiguousarray(np.asarray(x, np.float32).astype(BFNP))


def _build(causal: bool):
    nc = bacc.Bacc("TRN2", target_bir_lowering=False, debug=False,
                   num_devices=N_CORES)
    htb = nc.dram_tensor("htb", [KT_TILES, 128, S], BF16,
                         kind="ExternalInput")
    wqb = nc.dram_tensor("wqb", [KT_TILES, 128, HC], BF16,
                         kind="ExternalInput")
    wkb = nc.dram_tensor("wkb", [KT_TILES, 128, HC], BF16,
                         kind="ExternalInput")
    wvb = nc.dram_tensor("wvb", [KT_TILES, 128, HC], BF16,
                         kind="ExternalInput")
    wob = nc.dram_tensor("wob", [HPC, 128, H], BF16, kind="ExternalInput")
    cosb = nc.dram_tensor("cosb", [D, S], BF16, kind="ExternalInput")
    sinb = nc.dram_tensor("sinb", [D, S], BF16, kind="ExternalInput")
    if causal:
        mband = nc.dram_tensor("mband", [128, 896], BF16,
                               kind="ExternalInput")
    else:
        maskT = nc.dram_tensor("maskT", [S, S], F32, kind="ExternalInput")
    po = nc.dram_tensor("po", [H, S], BF16, kind="ExternalOutput")

    def mm(out, lhsT, rhs, start, stop):
        nc.tensor.matmul(out, lhsT, rhs, start=start, stop=stop)

    from contextlib import ExitStack
    with tile.TileContext(nc) as tc:
        at_pool_cm = tc.tile_pool(name="at", bufs=1)
        at_pool = at_pool_cm.__enter__()
        AT = at_pool.tile([128, HPC, S], BF16, tag="at", name="AT")

        es_res = ExitStack()
        kt_pool = es_res.enter_context(tc.tile_pool(name="kt", bufs=HPC))
        v_pool = es_res.enter_context(tc.tile_pool(name="v", bufs=SJT))
        wqk_pool = es_res.enter_context(
            tc.tile_pool(name="wqk", bufs=2 * KT_TILES))
        KT = [kt_pool.tile([128, S], BF16, tag="kt", name=f"KT{i}")
              for i in range(HPC)]
        V = [v_pool.tile([128, HC], BF16, tag="v", name=f"V{i}")
             for i in range(SJT)]
        WQ = [wqk_pool.tile([128, HC], BF16, tag="w", name=f"WQ{k}")
              for k in range(KT_TILES)]
        WK = [wqk_pool.tile([128, HC], BF16, tag="w", name=f"WK{k}")
              for k in range(KT_TILES)]
        for k in range(KT_TILES):
            nc.sync.dma_start(out=WQ[k][:], in_=wqb[k])
            nc.sync.dma_start(out=WK[k][:], in_=wkb[k])

        with tc.tile_pool(name="qtc", bufs=6) as qtp, \
             tc.tile_pool(name="ht", bufs=KT_TILES + 2) as htp, \
             tc.tile_pool(name="wvs", bufs=6) as wvp, \
             tc.tile_pool(name="cs", bufs=4) as csp, \
             tc.tile_pool(name="rope", bufs=2) as rp, \
             tc.tile_pool(name="aconst", bufs=1) as cpool, \
             tc.tile_pool(name="aes", bufs=3) as esp, \
             tc.tile_pool(name="am", bufs=1 if causal else 3) as mpool, \
             tc.tile_pool(name="ar", bufs=2) as rpool, \
             tc.tile_pool(name="mainps", bufs=4, space="PSUM") as psp:
            ones_col32 = cpool.tile([128, 1], F32, tag="oc32")
            nc.vector.memset(ones_col32[:], 1.0)
            ones_col = cpool.tile([128, 1], BF16, tag="oc")
            nc.vector.tensor_copy(ones_col[:], ones_col32[:])
            ones_row32 = cpool.tile([1, 128], F32, tag="or32")
            nc.vector.memset(ones_row32[:], 1.0)
            ones_row = cpool.tile([1, 128], F32R, tag="or")
            nc.vector.tensor_copy(ones_row[:], ones_row32[:])
            if causal:
                mb = cpool.tile([128, 896], BF16, tag="mb", name="mb")
                nc.sync.dma_start(out=mb[:], in_=mband[:, :])

            def rope_evict(ps, dst_ap, cosc, sinc):
                # dst = psum*cos + shift(psum)*sin_signed
                raw = rp.tile([128, CH], F32, tag="raw", name="raw")
                nc.scalar.copy(out=raw[:], in_=ps[:])
                shf = rp.tile([128, CH], F32, tag="shf", name="shf")
                nc.gpsimd.dma_start(out=shf[0:64, :], in_=raw[64:128, :])
                nc.gpsimd.dma_start(out=shf[64:128, :], in_=raw[0:64, :])
                tmp = rp.tile([128, CH], F32, tag="tmp", name="tmp")
                nc.vector.tensor_mul(tmp[:], shf[:], sinc[:])
                nc.vector.tensor_mul(dst_ap, raw[:], cosc[:])
                nc.vector.tensor_add(dst_ap, dst_ap, tmp[:])

            for c in range(NCH):
                cosc = csp.tile([128, CH], BF16, tag="cs", name="cosc")
                sinc = csp.tile([128, CH], BF16, tag="cs", name="sinc")
                nc.sync.dma_start(out=cosc[:], in_=cosb[:, bass.ts(c, CH)])
                nc.sync.dma_start(out=sinc[:], in_=sinb[:, bass.ts(c, CH)])
                hts = []
                for k in range(KT_TILES):
                    ht_t = htp.tile([128, CH], BF16, tag="ht", name="ht_t")
                    nc.sync.dma_start(out=ht_t[:],
                                      in_=htb[k][:, bass.ts(c, CH)])
                    hts.append(ht_t)
                # ---- Q pass ----
                QTc = [qtp.tile([128, CH], BF16, tag="qtc", name=f"QTc{i}")
                       for i in range(HPC)]
                qps = [psp.tile([128, CH], F32, tag="ps", name=f"qps{i}")
                       for i in range(HPC)]
                for k in range(KT_TILES):
                    st, sp = (k == 0), (k == KT_TILES - 1)
                    for d in range(HPC):
                        mm(qps[d][:], WQ[k][:, bass.ts(d, 128)], hts[k][:],
                           st, sp)
                for d in range(HPC):
                    rope_evict(qps[d], QTc[d][:], cosc, sinc)
                # ---- K pass ----
                kps = [psp.tile([128, CH], F32, tag="ps", name=f"kps{i}")
                       for i in range(HPC)]
                for k in range(KT_TILES):
                    st, sp = (k == 0), (k == KT_TILES - 1)
                    for d in range(HPC):
                        mm(kps[d][:], WK[k][:, bass.ts(d, 128)], hts[k][:],
                           st, sp)
                for d in range(HPC):
                    rope_evict(kps[d], KT[d][:, bass.ts(c, CH)], cosc, sinc)
                # ---- V pass (wv streamed) ----
                vps = [psp.tile([128, HC], F32, tag="ps", name=f"vps{i}")
                       for i in range(HPC)]
                for k in range(KT_TILES):
                    wv_t = wvp.tile([128, HC], BF16, tag="wv", name="wv_t")
                    nc.gpsimd.dma_start(out=wv_t[:], in_=wvb[k])
                    st, sp = (k == 0), (k == KT_TILES - 1)
                    for jl in range(4):
                        mm(vps[jl][:], hts[k][:, bass.ts(jl, 128)], wv_t[:],
                           st, sp)
                for jl in range(4):
                    nc.scalar.copy(out=V[4 * c + jl][:], in_=vps[jl][:])

                # ---- attention for i-chunk c (K/V chunks <= c) ----
                ic = c
                jmax = 4 * ic + 4 if causal else SJT
                for h in range(HPC):
                    sum_ps = psp.tile([1, CH], F32, tag="sum", bufs=2,
                                      name="sum_ps")
                    o_ps = psp.tile([128, CH], F32, tag="o", bufs=2,
                                    name="o_ps")
                    # software-pipelined: emit scores(j+1) before sum/PV(j)
                    es_list = []

                    def drain_one():
                        pj, pes = es_list.pop(0)
                        stq, spq = (pj == 0), (pj == jmax - 1)
                        mm(sum_ps[:], ones_col[:], pes[:], stq, spq)
                        mm(o_ps[:], V[pj][:, bass.ts(h, 128)], pes[:],
                           stq, spq)

                    for j in range(jmax):
                        s_ps = psp.tile([128, CH], F32, tag="ps",
                                        name="s_ps")
                        mm(s_ps[:], KT[h][:, bass.ts(j, 128)], QTc[h][:],
                           True, True)
                        if causal:
                            if j >= 4 * ic:
                                off = 384 - (j - 4 * ic) * 128
                                nc.vector.tensor_add(
                                    s_ps[:], s_ps[:], mb[:, off:off + CH])
                        else:
                            mt = mpool.tile([128, CH], F32, tag="mt",
                                            name="mt")
                            nc.sync.dma_start(
                                out=mt[:],
                                in_=maskT[bass.ts(j, 128), bass.ts(ic, CH)])
                            nc.vector.tensor_add(s_ps[:], s_ps[:], mt[:])
                        es_t = esp.tile([128, CH], BF16, tag="es",
                                        name="es_t")
                        nc.scalar.activation(es_t[:], s_ps[:], EXPF)
                        es_list.append((j, es_t))
                        if len(es_list) > 1:
                            drain_one()
                    drain_one()

                    rsum = rpool.tile([1, CH], F32, tag="rs", name="rsum")
                    rscr = rpool.tile([1, CH], F32, tag="rscr", name="rscr")
                    nc.vector.reciprocal_approx_accurate(
                        out=rsum[:], in_=sum_ps[:], scratch=rscr[:])
                    rsumr = rpool.tile([1, CH], F32R, tag="rsr",
                                       name="rsumr")
                    nc.vector.tensor_copy(rsumr[:], rsum[:])
                    b_ps = psp.tile([128, CH], F32, tag="ps", name="b_ps")
                    mm(b_ps[:], ones_row[:], rsumr[:], True, True)
                    rb = rpool.tile([128, CH], BF16, tag="rb", name="rb")
                    nc.scalar.copy(out=rb[:], in_=b_ps[:])
                    nc.vector.tensor_mul(
                        AT[:, h, bass.ts(ic, CH)], o_ps[:], rb[:])
        es_res.close()   # free weights/KT/V/ht SBUF before o_proj

        # ---------- o_proj  po = wo^T @ attnT ----------
        with tc.tile_pool(name="o_wo", bufs=HPC) as wop, \
             tc.tile_pool(name="o_out", bufs=4) as outp, \
             tc.tile_pool(name="o_ps", bufs=4, space="PSUM") as psp:
            WO = [wop.tile([128, H], BF16, tag="wo", name=f"WO{i}")
                  for i in range(HPC)]
            for kl in range(HPC):
                nc.sync.dma_start(out=WO[kl][:], in_=wob[kl])
            for ic in range(NCH):
                for n in range(H // 128):
                    pps = psp.tile([128, CH], F32, tag="ps", name="pps")
                    for kl in range(HPC):
                        mm(pps[:], WO[kl][:, bass.ts(n, 128)],
                           AT[:, kl, bass.ts(ic, CH)],
                           kl == 0, kl == HPC - 1)
                    ot = outp.tile([128, CH], BF16, tag="ot", name="ot")
                    nc.scalar.copy(out=ot[:], in_=pps[:])
                    nc.gpsimd.dma_start(
                        out=po[bass.ts(n, 128), bass.ts(ic, CH)], in_=ot[:])
        at_pool_cm.__exit__(None, None, None)
    nc.compile()
    return nc


_CACHE = {}


def _get_nc(causal):
    if causal not in _CACHE:
        _CACHE[causal] = _build(causal)
    return _CACHE[causal]


def kernel(hidden_states, attention_mask, position_ids, Wq, Wk, Wv, Wo):
    global last_exec_time_ns
    B, S_, H_ = hidden_states.shape
    assert (B, S_, H_) == (1, S, H)
    hs = np.asarray(hidden_states, dtype=np.float32)
    mask = np.asarray(attention_mask, dtype=np.float32)[0, 0]
    pos = np.asarray(position_ids)[0].astype(np.float64)

    # causal-mask fast path check
    iu = np.triu_indices(S, k=1)
    il = np.tril_indices(S, k=0)
    causal = bool(np.all(mask[il] == 0.0) and np.all(mask[iu] <= -1e30))

    hT = np.asarray(hs[0]).T               # [H, S]
    scale = 1.0 / np.sqrt(D)

    inv_freq = 1.0 / (ROPE_BASE ** (np.arange(0, D, 2, dtype=np.float64) / D))
    ang = pos[None, :] * np.concatenate([inv_freq, inv_freq])[:, None]  # [D,S]
    cosb = _b(np.cos(ang))
    sgn = np.ones((D, 1)); sgn[: D // 2] = -1.0
    sinb = _b(np.sin(ang) * sgn)

    htb = _b(hT).reshape(KT_TILES, 128, S)
    wq_s = _b(np.asarray(Wq, np.float64) * scale)
    wk_b = _b(Wk)
    wv_b = _b(Wv)
    wo_b = _b(Wo)

    if causal:
        # band mask tile [128, 896]: mb[r, y] = NEG iff r > y - 384
        rr = np.arange(128)[:, None]
        yy = np.arange(896)[None, :]
        mband = np.ascontiguousarray(
            np.where(rr > yy - 384, NEG, 0.0).astype(BFNP))
    else:
        maskT = _r(mask.T)

    nc = _get_nc(causal)
    in_maps = []
    for c in range(N_CORES):
        sl = slice(c * HC, (c + 1) * HC)
        m = {
            "htb": htb,
            "wqb": np.ascontiguousarray(wq_s[:, sl]).reshape(
                KT_TILES, 128, HC),
            "wkb": np.ascontiguousarray(wk_b[:, sl]).reshape(
                KT_TILES, 128, HC),
            "wvb": np.ascontiguousarray(wv_b[:, sl]).reshape(
                KT_TILES, 128, HC),
            "wob": np.ascontiguousarray(wo_b[sl, :]).reshape(HPC, 128, H),
            "cosb": cosb,
            "sinb": sinb,
        }
        if causal:
            m["mband"] = mband
        else:
            m["maskT"] = maskT
        in_maps.append(m)

    trace = bool(int(os.environ.get("BASS_KERNEL_TRACE", "0")))
    kw = {}
    if trace:
        kw["trace"] = True
        kw["tmpdir"] = os.environ.get("BASS_KERNEL_TRACE_DIR") or None
    res = run_bass_kernel_spmd(nc, in_maps, list(range(N_CORES)), **kw)
    last_exec_time_ns = res.exec_time_ns

    acc = np.zeros((H, S), dtype=np.float32)
    for c in range(N_CORES):
        acc += res.results[c]["po"].astype(np.float32)
    out = acc.T.reshape(1, S, H)
    return out


# revision 12
# speedup vs baseline: 1.3456x; 1.0991x over previous
"""Trainium2 Bass kernel for LlamaAttention (B=1, S=2048, H=4096, 32 heads).

Sharding: tensor-parallel over heads. 8 cores x 4 heads. Each core:
  - QKV projections in bf16 (1 cyc/out-col on the PE at 2.4 GHz; fp8
    DoubleRow measured at the same out-col rate, so bf16 wins once
    accuracy needs >1 fp8 pass). Wq/Wk resident in SBUF, Wv streamed.
  - RoPE on Q^T/K^T (rotate-half = partition swap via SBUF DMA).
  - causal attention in transposed layout (keys on partitions), bf16
    scores / exp / PV; j-tiles processed in pairs with one [128,1024]
    exp per pair; software-pipelined so exp (Act engine) overlaps the
    next pair's matmuls; fully-masked blocks skipped; softmax without
    max subtraction; column sums via ones-matmul; 1/sum broadcast via
    gpsimd partition_broadcast (no PSUM bank needed).
  - attention output kept in SBUF (bf16); o_proj bf16 with WO loads
    overlapped into the last attention chunk; partial po written bf16.
  Host sums the 8 partials and transposes. No collectives.
"""

import os
import sys

if "/opt/trn_rl_repo" not in sys.path:
    sys.path.insert(0, "/opt/trn_rl_repo")

import numpy as np
import ml_dtypes

from concourse import bacc, mybir, tile
from concourse import bass
from concourse.bass_utils import run_bass_kernel_spmd

F32 = mybir.dt.float32
F32R = mybir.dt.float32r
BF16 = mybir.dt.bfloat16
EXPF = mybir.ActivationFunctionType.Exp

N_CORES = 8
S = 2048
H = 4096
N_HEADS = 32
D = 128                      # head dim
HPC = N_HEADS // N_CORES     # heads per core = 4
HC = HPC * D                 # per-core hidden slice = 512
CH = 512                     # seq chunk width
NCH = S // CH                # 4 chunks
KT_TILES = H // 128          # 32 contraction tiles for projections
SJT = S // 128               # 16 seq j-tiles
ROPE_BASE = 10000.0
NEG = -1.0e9

BFNP = ml_dtypes.bfloat16

last_exec_time_ns = None


def _r(x):
    return np.ascontiguousarray(x, dtype=np.float32)


def _b(x):
    return np.ascontiguousarray(np.asarray(x, np.float32).astype(BFNP))


def _build(causal: bool):
    nc = bacc.Bacc("TRN2", target_bir_lowering=False, debug=False,
                   num_devices=N_CORES)
    htb = nc.dram_tensor("htb", [KT_TILES, 128, S], BF16,
                         kind="ExternalInput")
    wqb = nc.dram_tensor("wqb", [KT_TILES, 128, HC], BF16,
                         kind="ExternalInput")
    wkb = nc.dram_tensor("wkb", [KT_TILES, 128, HC], BF16,
                         kind="ExternalInput")
    wvb = nc.dram_tensor("wvb", [KT_TILES, 128, HC], BF16,
                         kind="ExternalInput")
    wob = nc.dram_tensor("wob", [HPC, 128, H], BF16, kind="ExternalInput")
    cosb = nc.dram_tensor("cosb", [D, S], BF16, kind="ExternalInput")
    sinb = nc.dram_tensor("sinb", [D, S], BF16, kind="ExternalInput")
    if causal:
        mband = nc.dram_tensor("mband", [128, 896], BF16,
                               kind="ExternalInput")
    else:
        maskT = nc.dram_tensor("maskT", [S, S], F32, kind="ExternalInput")
    po = nc.dram_tensor("po", [H, S], BF16, kind="ExternalOutput")

    def mm(out, lhsT, rhs, start, stop):
        nc.tensor.matmul(out, lhsT, rhs, start=start, stop=stop)

    from contextlib import ExitStack
    with tile.TileContext(nc) as tc:
        at_pool_cm = tc.tile_pool(name="at", bufs=1)
        at_pool = at_pool_cm.__enter__()
        AT = at_pool.tile([128, HPC, S], BF16, tag="at", name="AT")

        es_res = ExitStack()
        kt_pool = es_res.enter_context(tc.tile_pool(name="kt", bufs=HPC))
        v_pool = es_res.enter_context(tc.tile_pool(name="v", bufs=SJT))
        wqk_pool = es_res.enter_context(
            tc.tile_pool(name="wqk", bufs=2 * KT_TILES))
        KT = [kt_pool.tile([128, S], BF16, tag="kt", name=f"KT{i}")
              for i in range(HPC)]
        V = [v_pool.tile([128, HC], BF16, tag="v", name=f"V{i}")
             for i in range(SJT)]
        WQ = [wqk_pool.tile([128, HC], BF16, tag="w", name=f"WQ{k}")
              for k in range(KT_TILES)]
        WK = [wqk_pool.tile([128, HC], BF16, tag="w", name=f"WK{k}")
              for k in range(KT_TILES)]
        for k in range(KT_TILES):
            nc.sync.dma_start(out=WQ[k][:], in_=wqb[k])
            nc.sync.dma_start(out=WK[k][:], in_=wkb[k])

        with tc.tile_pool(name="qtc", bufs=4) as qtp, \
             tc.tile_pool(name="ht", bufs=KT_TILES) as htp, \
             tc.tile_pool(name="wvs", bufs=6) as wvp, \
             tc.tile_pool(name="rope", bufs=2) as rp, \
             tc.tile_pool(name="aconst", bufs=1) as cpool, \
             tc.tile_pool(name="aes", bufs=2) as esp, \
             tc.tile_pool(name="am", bufs=1 if causal else 4) as mpool, \
             tc.tile_pool(name="ar", bufs=2) as rpool, \
             tc.tile_pool(name="mainps", bufs=2, space="PSUM") as psp:
            ones_col32 = cpool.tile([128, 1], F32, tag="oc32")
            nc.vector.memset(ones_col32[:], 1.0)
            ones_col = cpool.tile([128, 1], BF16, tag="oc")
            nc.vector.tensor_copy(ones_col[:], ones_col32[:])
            cosT = cpool.tile([128, S], BF16, tag="cos", name="cosT")
            sinT = cpool.tile([128, S], BF16, tag="sin", name="sinT")
            nc.sync.dma_start(out=cosT[:], in_=cosb[:, :])
            nc.sync.dma_start(out=sinT[:], in_=sinb[:, :])
            if causal:
                mb = cpool.tile([128, 896], BF16, tag="mb", name="mb")
                nc.sync.dma_start(out=mb[:], in_=mband[:, :])

            def rope_evict(ps, dst_ap, c):
                # dst = psum*cos + shift(psum)*sin_signed
                cosc = cosT[:, bass.ts(c, CH)]
                sinc = sinT[:, bass.ts(c, CH)]
                raw = rp.tile([128, CH], BF16, tag="raw", name="raw")
                nc.scalar.copy(out=raw[:], in_=ps[:])
                shf = rp.tile([128, CH], BF16, tag="shf", name="shf")
                nc.gpsimd.dma_start(out=shf[0:64, :], in_=raw[64:128, :])
                nc.gpsimd.dma_start(out=shf[64:128, :], in_=raw[0:64, :])
                tmp = rp.tile([128, CH], BF16, tag="tmp", name="tmp")
                nc.vector.tensor_mul(tmp[:], shf[:], sinc)
                nc.vector.tensor_mul(dst_ap, raw[:], cosc)
                nc.vector.tensor_add(dst_ap, dst_ap, tmp[:])

            def load_ht(c):
                hts = []
                for k in range(KT_TILES):
                    ht_t = htp.tile([128, CH], BF16, tag="ht", name="ht_t")
                    nc.sync.dma_start(out=ht_t[:],
                                      in_=htb[k][:, bass.ts(c, CH)])
                    hts.append(ht_t)
                return hts

            hts = load_ht(0)
            for c in range(NCH):
                # ---- Q pass (head-dim pairs, early evict) ----
                QTc = [qtp.tile([128, CH], BF16, tag="qtc", name=f"QTc{i}")
                       for i in range(HPC)]
                for dp in range(2):
                    qp2 = psp.tile([128, 2, CH], F32, tag="big",
                                   name="qp2")
                    for k in range(KT_TILES):
                        st, sp = (k == 0), (k == KT_TILES - 1)
                        for t in range(2):
                            d = 2 * dp + t
                            mm(qp2[:, t, :], WQ[k][:, bass.ts(d, 128)],
                               hts[k][:], st, sp)
                    for t in range(2):
                        rope_evict(qp2[:, t, :], QTc[2 * dp + t][:], c)
                # ---- K pass ----
                for dp in range(2):
                    kp2 = psp.tile([128, 2, CH], F32, tag="big",
                                   name="kp2")
                    for k in range(KT_TILES):
                        st, sp = (k == 0), (k == KT_TILES - 1)
                        for t in range(2):
                            d = 2 * dp + t
                            mm(kp2[:, t, :], WK[k][:, bass.ts(d, 128)],
                               hts[k][:], st, sp)
                    for t in range(2):
                        rope_evict(kp2[:, t, :],
                                   KT[2 * dp + t][:, bass.ts(c, CH)], c)
                # ---- V pass (wv streamed, both jl-pairs live) ----
                vp2 = [psp.tile([128, 2, CH], F32, tag="big",
                                name=f"vp2_{p}") for p in range(2)]
                for k in range(KT_TILES):
                    wv_t = wvp.tile([128, HC], BF16, tag="wv", name="wv_t")
                    nc.gpsimd.dma_start(out=wv_t[:], in_=wvb[k])
                    st, sp = (k == 0), (k == KT_TILES - 1)
                    for jl in range(4):
                        mm(vp2[jl // 2][:, jl % 2, :],
                           hts[k][:, bass.ts(jl, 128)], wv_t[:], st, sp)
                for jl in range(4):
                    nc.scalar.copy(out=V[4 * c + jl][:],
                                   in_=vp2[jl // 2][:, jl % 2, :])

                # prefetch next chunk's hidden tiles during attention
                if c + 1 < NCH:
                    hts = load_ht(c + 1)

                # ---- attention for i-chunk c (K/V chunks <= c) ----
                ic = c
                jp_max = (2 * ic + 2) if causal else (SJT // 2)
                for h in range(HPC):
                    sum_ps = psp.tile([1, CH], F32, tag="sum", bufs=2,
                                      name="sum_ps")
                    o_ps = psp.tile([128, CH], F32, tag="o", bufs=2,
                                    name="o_ps")
                    # pair-pipelined: exp of pair p overlaps matmuls of p+1
                    pend = []

                    def drain():
                        jp_, es2_ = pend.pop(0)
                        for t in range(2):
                            j = 2 * jp_ + t
                            stq = (j == 0)
                            spq = (j == 2 * jp_max - 1)
                            mm(sum_ps[:], ones_col[:], es2_[:, t, :],
                               stq, spq)
                            mm(o_ps[:], V[j][:, bass.ts(h, 128)],
                               es2_[:, t, :], stq, spq)

                    for jp in range(jp_max):
                        s2 = psp.tile([128, 2, CH], F32, tag="big",
                                      name="s2")
                        for t in range(2):
                            j = 2 * jp + t
                            mm(s2[:, t, :], KT[h][:, bass.ts(j, 128)],
                               QTc[h][:], True, True)
                            if causal:
                                if j >= 4 * ic:
                                    off = 384 - (j - 4 * ic) * 128
                                    nc.vector.tensor_add(
                                        s2[:, t, :], s2[:, t, :],
                                        mb[:, off:off + CH])
                            else:
                                mt = mpool.tile([128, CH], F32, tag="mt",
                                                name="mt")
                                nc.sync.dma_start(
                                    out=mt[:],
                                    in_=maskT[bass.ts(j, 128),
                                              bass.ts(ic, CH)])
                                nc.vector.tensor_add(s2[:, t, :],
                                                     s2[:, t, :], mt[:])
                        es2 = esp.tile([128, 2, CH], BF16, tag="es",
                                       name="es2")
                        nc.scalar.activation(es2[:], s2[:], EXPF)
                        pend.append((jp, es2))
                        if len(pend) > 1:
                            drain()
                    drain()

                    rsum = rpool.tile([1, CH], F32, tag="rs", name="rsum")
                    rscr = rpool.tile([1, CH], F32, tag="rscr", name="rscr")
                    nc.vector.reciprocal_approx_accurate(
                        out=rsum[:], in_=sum_ps[:], scratch=rscr[:])
                    rb = rpool.tile([128, CH], F32, tag="rb", name="rb")
                    nc.gpsimd.partition_broadcast(rb[:], rsum[:])
                    nc.vector.tensor_mul(
                        AT[:, h, bass.ts(ic, CH)], o_ps[:], rb[:])
        es_res.close()   # free weights/KT/V/ht SBUF before o_proj

        # ---------- o_proj  po = wo^T @ attnT ----------
        with tc.tile_pool(name="o_wo", bufs=HPC) as wop, \
             tc.tile_pool(name="o_out", bufs=8) as outp, \
             tc.tile_pool(name="o_ps", bufs=8, space="PSUM") as psp:
            WO = [wop.tile([128, H], BF16, tag="wo", name=f"WO{i}")
                  for i in range(HPC)]
            for kl in range(HPC):
                nc.sync.dma_start(out=WO[kl][:], in_=wob[kl])
            NB = 4           # n-tiles per block; kl-outer within a block
            for ic in range(NCH):
                for nb in range(H // 128 // NB):
                    pps = [psp.tile([128, CH], F32, tag="ps", name="pps")
                           for _ in range(NB)]
                    for kl in range(HPC):
                        for i in range(NB):
                            n = nb * NB + i
                            mm(pps[i][:], WO[kl][:, bass.ts(n, 128)],
                               AT[:, kl, bass.ts(ic, CH)],
                               kl == 0, kl == HPC - 1)
                    for i in range(NB):
                        n = nb * NB + i
                        ot = outp.tile([128, CH], BF16, tag="ot", name="ot")
                        nc.scalar.copy(out=ot[:], in_=pps[i][:])
                        nc.gpsimd.dma_start(
                            out=po[bass.ts(n, 128), bass.ts(ic, CH)],
                            in_=ot[:])
        at_pool_cm.__exit__(None, None, None)
    nc.compile()
    return nc


_CACHE = {}


def _get_nc(causal):
    if causal not in _CACHE:
        _CACHE[causal] = _build(causal)
    return _CACHE[causal]


def kernel(hidden_states, attention_mask, position_ids, Wq, Wk, Wv, Wo):
    global last_exec_time_ns
    B, S_, H_ = hidden_states.shape
    assert (B, S_, H_) == (1, S, H)
    hs = np.asarray(hidden_states, dtype=np.float32)
    mask = np.asarray(attention_mask, dtype=np.float32)[0, 0]
    pos = np.asarray(position_ids)[0].astype(np.float64)

    # causal-mask fast path check
    iu = np.triu_indices(S, k=1)
    il = np.tril_indices(S, k=0)
    causal = bool(np.all(mask[il] == 0.0) and np.all(mask[iu] <= -1e30))

    hT = np.asarray(hs[0]).T               # [H, S]
    scale = 1.0 / np.sqrt(D)

    inv_freq = 1.0 / (ROPE_BASE ** (np.arange(0, D, 2, dtype=np.float64) / D))
    ang = pos[None, :] * np.concatenate([inv_freq, inv_freq])[:, None]  # [D,S]
    cosb = _b(np.cos(ang))
    sgn = np.ones((D, 1)); sgn[: D // 2] = -1.0
    sinb = _b(np.sin(ang) * sgn)

    htb = _b(hT).reshape(KT_TILES, 128, S)
    wq_s = _b(np.asarray(Wq, np.float64) * scale)
    wk_b = _b(Wk)
    wv_b = _b(Wv)
    wo_b = _b(Wo)

    if causal:
        # band mask tile [128, 896]: mb[r, y] = NEG iff r > y - 384
        rr = np.arange(128)[:, None]
        yy = np.arange(896)[None, :]
        mband = np.ascontiguousarray(
            np.where(rr > yy - 384, NEG, 0.0).astype(BFNP))
    else:
        maskT = _r(mask.T)

    nc = _get_nc(causal)
    in_maps = []
    for c in range(N_CORES):
        sl = slice(c * HC, (c + 1) * HC)
        m = {
            "htb": htb,
            "wqb": np.ascontiguousarray(wq_s[:, sl]).reshape(
                KT_TILES, 128, HC),
            "wkb": np.ascontiguousarray(wk_b[:, sl]).reshape(
                KT_TILES, 128, HC),
            "wvb": np.ascontiguousarray(wv_b[:, sl]).reshape(
                KT_TILES, 128, HC),
            "wob": np.ascontiguousarray(wo_b[sl, :]).reshape(HPC, 128, H),
            "cosb": cosb,
            "sinb": sinb,
        }
        if causal:
            m["mband"] = mband
        else:
            m["maskT"] = maskT
        in_maps.append(m)

    trace = bool(int(os.environ.get("BASS_KERNEL_TRACE", "0")))
    kw = {}
    if trace:
        kw["trace"] = True
        kw["tmpdir"] = os.environ.get("BASS_KERNEL_TRACE_DIR") or None
    res = run_bass_kernel_spmd(nc, in_maps, list(range(N_CORES)), **kw)
    last_exec_time_ns = res.exec_time_ns

    acc = np.zeros((H, S), dtype=np.float32)
    for c in range(N_CORES):
        acc += res.results[c]["po"].astype(np.float32)
    out = acc.T.reshape(1, S, H)
    return out


# revision 17
# speedup vs baseline: 1.3816x; 1.0267x over previous
"""Trainium2 Bass kernel for LlamaAttention (B=1, S=2048, H=4096, 32 heads).

Sharding: tensor-parallel over heads. 8 cores x 4 heads. Each core:
  - QKV projections in bf16 (1 cyc/out-col on the PE at 2.4 GHz; fp8
    DoubleRow measured at the same out-col rate, so bf16 wins once
    accuracy needs >1 fp8 pass). Wq/Wk resident in SBUF as single
    p-major tiles (few large DMAs; packets of one DMA spread over all
    16 DMA engines), Wv streamed per chunk.
  - head-dim PAIR accumulation in [128,2,CH] PSUM tiles (2 banks) with
    early eviction, so Q/K/V pass transitions don't stall; RoPE on
    Q^T/K^T (rotate-half = partition swap via SBUF DMA).
  - causal attention in transposed layout (keys on partitions), bf16
    scores / exp / PV; j-tile pairs share one [128,1024] exp;
    globally software-pipelined across heads so exp always overlaps
    matmuls; fully-masked blocks skipped; softmax without max
    subtraction; column sums via ones-matmul; 1/sum broadcast via
    gpsimd partition_broadcast (no PSUM bank).
  - attention output kept in SBUF (bf16); o_proj bf16, WO load
    overlapped into the last attention chunk; partial po written bf16.
  Host sums the 8 partials and transposes. No collectives.
"""

import os
import sys

if "/opt/trn_rl_repo" not in sys.path:
    sys.path.insert(0, "/opt/trn_rl_repo")

import numpy as np
import ml_dtypes

from concourse import bacc, mybir, tile
from concourse import bass
from concourse.bass_utils import run_bass_kernel_spmd

F32 = mybir.dt.float32
F32R = mybir.dt.float32r
BF16 = mybir.dt.bfloat16
EXPF = mybir.ActivationFunctionType.Exp

N_CORES = 8
S = 2048
H = 4096
N_HEADS = 32
D = 128                      # head dim
HPC = N_HEADS // N_CORES     # heads per core = 4
HC = HPC * D                 # per-core hidden slice = 512
CH = 512                     # seq chunk width
NCH = S // CH                # 4 chunks
KT_TILES = H // 128          # 32 contraction tiles for projections
SJT = S // 128               # 16 seq j-tiles
ROPE_BASE = 10000.0
NEG = -1.0e9

BFNP = ml_dtypes.bfloat16

last_exec_time_ns = None


def _r(x):
    return np.ascontiguousarray(x, dtype=np.float32)


def _b(x):
    return np.ascontiguousarray(np.asarray(x, np.float32).astype(BFNP))


def _pmajor(a, kt):
    """[kt*128, C] -> [128, kt, C] partition-major layout."""
    R, C = a.shape
    return np.ascontiguousarray(
        np.asarray(a).reshape(kt, 128, C).transpose(1, 0, 2))


def _build(causal: bool):
    nc = bacc.Bacc("TRN2", target_bir_lowering=False, debug=False,
                   num_devices=N_CORES)
    htb = nc.dram_tensor("htb", [128, KT_TILES, S], BF16,
                         kind="ExternalInput")
    wqb = nc.dram_tensor("wqb", [128, KT_TILES, HC], BF16,
                         kind="ExternalInput")
    wkb = nc.dram_tensor("wkb", [128, KT_TILES, HC], BF16,
                         kind="ExternalInput")
    wvb = nc.dram_tensor("wvb", [KT_TILES, 128, HC], BF16,
                         kind="ExternalInput")
    wob = nc.dram_tensor("wob", [128, HPC, H], BF16, kind="ExternalInput")
    cosb = nc.dram_tensor("cosb", [D, S], BF16, kind="ExternalInput")
    sinb = nc.dram_tensor("sinb", [D, S], BF16, kind="ExternalInput")
    if causal:
        mband = nc.dram_tensor("mband", [128, 896], BF16,
                               kind="ExternalInput")
    else:
        maskT = nc.dram_tensor("maskT", [S, S], F32, kind="ExternalInput")
    po = nc.dram_tensor("po", [H, S], BF16, kind="ExternalOutput")

    def mm(out, lhsT, rhs, start, stop):
        nc.tensor.matmul(out, lhsT, rhs, start=start, stop=stop)

    from contextlib import ExitStack
    with tile.TileContext(nc) as tc:
        at_pool_cm = tc.tile_pool(name="at", bufs=1)
        at_pool = at_pool_cm.__enter__()
        AT = at_pool.tile([128, HPC, S], BF16, tag="at", name="AT")

        es_res = ExitStack()
        kt_pool = es_res.enter_context(tc.tile_pool(name="kt", bufs=HPC))
        v_pool = es_res.enter_context(tc.tile_pool(name="v", bufs=SJT))
        wqk_pool = es_res.enter_context(tc.tile_pool(name="wqk", bufs=1))
        KT = [kt_pool.tile([128, S], BF16, tag="kt", name=f"KT{i}")
              for i in range(HPC)]
        V = [v_pool.tile([128, HC], BF16, tag="v", name=f"V{i}")
             for i in range(SJT)]
        WQ = wqk_pool.tile([128, KT_TILES, HC], BF16, tag="wq", name="WQ")
        WK = wqk_pool.tile([128, KT_TILES, HC], BF16, tag="wk", name="WK")
        # split loads in groups of 4 k-tiles: trickle-feeds the Q pass
        for g in range(8):
            nc.sync.dma_start(out=WQ[:, bass.ts(g, 4), :],
                              in_=wqb[:, bass.ts(g, 4), :])
        for g in range(8):
            nc.scalar.dma_start(out=WK[:, bass.ts(g, 4), :],
                                in_=wkb[:, bass.ts(g, 4), :])

        with tc.tile_pool(name="qtc", bufs=4) as qtp, \
             tc.tile_pool(name="ht", bufs=1) as htp, \
             tc.tile_pool(name="wvs", bufs=6) as wvp, \
             tc.tile_pool(name="rope", bufs=2) as rp, \
             tc.tile_pool(name="aconst", bufs=1) as cpool, \
             tc.tile_pool(name="aes", bufs=2) as esp, \
             tc.tile_pool(name="am", bufs=1 if causal else 4) as mpool, \
             tc.tile_pool(name="ar", bufs=1) as rpool, \
             tc.tile_pool(name="mainps", bufs=2, space="PSUM") as psp:
            ones_col32 = cpool.tile([128, 1], F32, tag="oc32")
            nc.vector.memset(ones_col32[:], 1.0)
            ones_col = cpool.tile([128, 1], BF16, tag="oc")
            nc.vector.tensor_copy(ones_col[:], ones_col32[:])
            cosT = cpool.tile([128, S], BF16, tag="cos", name="cosT")
            sinT = cpool.tile([128, S], BF16, tag="sin", name="sinT")
            nc.scalar.dma_start(out=cosT[:], in_=cosb[:, :])
            nc.scalar.dma_start(out=sinT[:], in_=sinb[:, :])
            if causal:
                mb = cpool.tile([128, 896], BF16, tag="mb", name="mb")
                nc.scalar.dma_start(out=mb[:], in_=mband[:, :])

            def rope_evict(ps, dst_ap, c):
                # dst = psum*cos + shift(psum)*sin_signed
                cosc = cosT[:, bass.ts(c, CH)]
                sinc = sinT[:, bass.ts(c, CH)]
                raw = rp.tile([128, CH], BF16, tag="raw", name="raw")
                nc.scalar.copy(out=raw[:], in_=ps)
                shf = rp.tile([128, CH], BF16, tag="shf", name="shf")
                nc.gpsimd.dma_start(out=shf[0:64, :], in_=raw[64:128, :])
                nc.gpsimd.dma_start(out=shf[64:128, :], in_=raw[0:64, :])
                tmp = rp.tile([128, CH], BF16, tag="tmp", name="tmp")
                nc.vector.tensor_mul(tmp[:], shf[:], sinc)
                nc.vector.tensor_mul(dst_ap, raw[:], cosc)
                nc.vector.tensor_add(dst_ap, dst_ap, tmp[:])

            def load_ht(c):
                ht_t = htp.tile([128, KT_TILES, CH], BF16, tag="ht",
                                name="ht_t")
                for g in range(8):
                    nc.sync.dma_start(out=ht_t[:, bass.ts(g, 4), :],
                                      in_=htb[:, bass.ts(g, 4),
                                              bass.ts(c, CH)])
                return ht_t

            HT = load_ht(0)
            for c in range(NCH):
                # ---- Q pass (head-dim pairs, early evict) ----
                QTc = [qtp.tile([128, CH], BF16, tag="qtc", name=f"QTc{i}")
                       for i in range(HPC)]
                for dp in range(2):
                    qp2 = psp.tile([128, 2, CH], F32, tag="big",
                                   name="qp2")
                    for k in range(KT_TILES):
                        st, sp = (k == 0), (k == KT_TILES - 1)
                        for t in range(2):
                            d = 2 * dp + t
                            mm(qp2[:, t, :], WQ[:, k, bass.ts(d, 128)],
                               HT[:, k, :], st, sp)
                    for t in range(2):
                        rope_evict(qp2[:, t, :], QTc[2 * dp + t][:], c)
                # ---- K pass ----
                for dp in range(2):
                    kp2 = psp.tile([128, 2, CH], F32, tag="big",
                                   name="kp2")
                    for k in range(KT_TILES):
                        st, sp = (k == 0), (k == KT_TILES - 1)
                        for t in range(2):
                            d = 2 * dp + t
                            mm(kp2[:, t, :], WK[:, k, bass.ts(d, 128)],
                               HT[:, k, :], st, sp)
                    for t in range(2):
                        rope_evict(kp2[:, t, :],
                                   KT[2 * dp + t][:, bass.ts(c, CH)], c)
                # ---- V pass (wv streamed, both jl-pairs live) ----
                vp2 = [psp.tile([128, 2, CH], F32, tag="big",
                                name=f"vp2_{p}") for p in range(2)]
                for k in range(KT_TILES):
                    wv_t = wvp.tile([128, HC], BF16, tag="wv", name="wv_t")
                    nc.gpsimd.dma_start(out=wv_t[:], in_=wvb[k])
                    st, sp = (k == 0), (k == KT_TILES - 1)
                    for jl in range(4):
                        mm(vp2[jl // 2][:, jl % 2, :],
                           HT[:, k, bass.ts(jl, 128)], wv_t[:], st, sp)
                for jl in range(4):
                    nc.scalar.copy(out=V[4 * c + jl][:],
                                   in_=vp2[jl // 2][:, jl % 2, :])

                # prefetch next chunk's hidden tile during attention
                if c + 1 < NCH:
                    HT = load_ht(c + 1)

                # ---- attention for i-chunk c (K/V chunks <= c) ----
                ic = c
                jp_max = (2 * ic + 2) if causal else (SJT // 2)

                def finish_head(h_, sum_, o_):
                    rsum = rpool.tile([1, CH], F32, tag="rs", name="rsum")
                    rscr = rpool.tile([1, CH], F32, tag="rscr",
                                      name="rscr")
                    nc.vector.reciprocal_approx_accurate(
                        out=rsum[:], in_=sum_[:], scratch=rscr[:])
                    rb = rpool.tile([128, CH], F32, tag="rb", name="rb")
                    nc.gpsimd.partition_broadcast(rb[:], rsum[:])
                    nc.vector.tensor_mul(
                        AT[:, h_, bass.ts(ic, CH)], o_[:], rb[:])

                pend = []

                def drain_one():
                    h_, jp_, es2_, sum_, o_ = pend.pop(0)
                    last = False
                    for t in range(2):
                        j = 2 * jp_ + t
                        stq = (j == 0)
                        last = (j == 2 * jp_max - 1)
                        mm(sum_[:], ones_col[:], es2_[:, t, :], stq, last)
                        mm(o_[:], V[j][:, bass.ts(h_, 128)], es2_[:, t, :],
                           stq, last)
                    if last:
                        finish_head(h_, sum_, o_)

                for h in range(HPC):
                    sum_ps = psp.tile([1, CH], F32, tag="sum", bufs=2,
                                      name="sum_ps")
                    o_ps = psp.tile([128, CH], F32, tag="o", bufs=2,
                                    name="o_ps")
                    for jp in range(jp_max):
                        s2 = psp.tile([128, 2, CH], F32, tag="big",
                                      name="s2")
                        for t in range(2):
                            j = 2 * jp + t
                            mm(s2[:, t, :], KT[h][:, bass.ts(j, 128)],
                               QTc[h][:], True, True)
                            if causal:
                                if j >= 4 * ic:
                                    off = 384 - (j - 4 * ic) * 128
                                    nc.vector.tensor_add(
                                        s2[:, t, :], s2[:, t, :],
                                        mb[:, off:off + CH])
                            else:
                                mt = mpool.tile([128, CH], F32, tag="mt",
                                                name="mt")
                                nc.sync.dma_start(
                                    out=mt[:],
                                    in_=maskT[bass.ts(j, 128),
                                              bass.ts(ic, CH)])
                                nc.vector.tensor_add(s2[:, t, :],
                                                     s2[:, t, :], mt[:])
                        es2 = esp.tile([128, 2, CH], BF16, tag="es",
                                       name="es2")
                        nc.scalar.activation(es2[:], s2[:], EXPF)
                        pend.append((h, jp, es2, sum_ps, o_ps))
                        if len(pend) > 1:
                            drain_one()
                while pend:
                    drain_one()
        es_res.close()   # free weights/KT/V/ht SBUF before o_proj

        # ---------- o_proj  po = wo^T @ attnT ----------
        with tc.tile_pool(name="o_wo", bufs=1) as wop, \
             tc.tile_pool(name="o_out", bufs=8) as outp, \
             tc.tile_pool(name="o_ps", bufs=8, space="PSUM") as psp:
            WOa = wop.tile([128, HPC, H], BF16, tag="wo", name="WOa")
            for kl in range(HPC):
                nc.sync.dma_start(out=WOa[:, kl, :], in_=wob[:, kl, :])
            NB = 4           # n-tiles per block; kl-outer within a block
            for ic in range(NCH):
                for nb in range(H // 128 // NB):
                    pps = [psp.tile([128, CH], F32, tag="ps", name="pps")
                           for _ in range(NB)]
                    for kl in range(HPC):
                        for i in range(NB):
                            n = nb * NB + i
                            mm(pps[i][:], WOa[:, kl, bass.ts(n, 128)],
                               AT[:, kl, bass.ts(ic, CH)],
                               kl == 0, kl == HPC - 1)
                    for i in range(NB):
                        n = nb * NB + i
                        ot = outp.tile([128, CH], BF16, tag="ot", name="ot")
                        nc.scalar.copy(out=ot[:], in_=pps[i][:])
                        nc.gpsimd.dma_start(
                            out=po[bass.ts(n, 128), bass.ts(ic, CH)],
                            in_=ot[:])
        at_pool_cm.__exit__(None, None, None)
    nc.compile()
    return nc


_CACHE = {}


def _get_nc(causal):
    if causal not in _CACHE:
        _CACHE[causal] = _build(causal)
    return _CACHE[causal]


def kernel(hidden_states, attention_mask, position_ids, Wq, Wk, Wv, Wo):
    global last_exec_time_ns
    B, S_, H_ = hidden_states.shape
    assert (B, S_, H_) == (1, S, H)
    hs = np.asarray(hidden_states, dtype=np.float32)
    mask = np.asarray(attention_mask, dtype=np.float32)[0, 0]
    pos = np.asarray(position_ids)[0].astype(np.float64)

    # causal-mask fast path check
    iu = np.triu_indices(S, k=1)
    il = np.tril_indices(S, k=0)
    causal = bool(np.all(mask[il] == 0.0) and np.all(mask[iu] <= -1e30))

    hT = np.asarray(hs[0]).T               # [H, S]
    scale = 1.0 / np.sqrt(D)

    inv_freq = 1.0 / (ROPE_BASE ** (np.arange(0, D, 2, dtype=np.float64) / D))
    ang = pos[None, :] * np.concatenate([inv_freq, inv_freq])[:, None]  # [D,S]
    cosb = _b(np.cos(ang))
    sgn = np.ones((D, 1)); sgn[: D // 2] = -1.0
    sinb = _b(np.sin(ang) * sgn)

    htb = _pmajor(_b(hT), KT_TILES)
    wq_s = _b(np.asarray(Wq, np.float64) * scale)
    wk_b = _b(Wk)
    wv_b = _b(Wv)
    wo_b = _b(Wo)

    if causal:
        # band mask tile [128, 896]: mb[r, y] = NEG iff r > y - 384
        rr = np.arange(128)[:, None]
        yy = np.arange(896)[None, :]
        mband = np.ascontiguousarray(
            np.where(rr > yy - 384, NEG, 0.0).astype(BFNP))
    else:
        maskT = _r(mask.T)

    nc = _get_nc(causal)
    in_maps = []
    for c in range(N_CORES):
        sl = slice(c * HC, (c + 1) * HC)
        m = {
            "htb": htb,
            "wqb": _pmajor(wq_s[:, sl], KT_TILES),
            "wkb": _pmajor(wk_b[:, sl], KT_TILES),
            "wvb": np.ascontiguousarray(wv_b[:, sl]).reshape(
                KT_TILES, 128, HC),
            "wob": _pmajor(wo_b[sl, :], HPC),
            "cosb": cosb,
            "sinb": sinb,
        }
        if causal:
            m["mband"] = mband
        else:
            m["maskT"] = maskT
        in_maps.append(m)

    trace = bool(int(os.environ.get("BASS_KERNEL_TRACE", "0")))
    kw = {}
    if trace:
        kw["trace"] = True
        kw["tmpdir"] = os.environ.get("BASS_KERNEL_TRACE_DIR") or None
    res = run_bass_kernel_spmd(nc, in_maps, list(range(N_CORES)), **kw)
    last_exec_time_ns = res.exec_time_ns

    acc = np.zeros((H, S), dtype=np.float32)
    for c in range(N_CORES):
        acc += res.results[c]["po"].astype(np.float32)
    out = acc.T.reshape(1, S, H)
    return out


# revision 20
# speedup vs baseline: 1.4046x; 1.0166x over previous
"""Trainium2 Bass kernel for LlamaAttention (B=1, S=2048, H=4096, 32 heads).

Sharding: tensor-parallel over heads. 8 cores x 4 heads. Each core:
  - QKV projections in bf16 (1 cyc/out-col on the PE at 2.4 GHz; fp8
    DoubleRow measured at the same out-col rate, so bf16 wins once
    accuracy needs >1 fp8 pass). Wq/Wk resident in SBUF as single
    p-major tiles (few large DMAs; packets of one DMA spread over all
    16 DMA engines), Wv streamed per chunk.
  - head-dim PAIR accumulation in [128,2,CH] PSUM tiles (2 banks) with
    early eviction, so Q/K/V pass transitions don't stall; RoPE on
    Q^T/K^T (rotate-half = partition swap via SBUF DMA).
  - causal attention in transposed layout (keys on partitions), bf16
    scores / exp / PV; j-tile pairs share one [128,1024] exp;
    globally software-pipelined across heads so exp always overlaps
    matmuls; fully-masked blocks skipped; softmax without max
    subtraction; column sums via ones-matmul; 1/sum broadcast via
    gpsimd partition_broadcast (no PSUM bank).
  - attention output kept in SBUF (bf16); o_proj bf16, WO load
    overlapped into the last attention chunk; partial po written bf16.
  Host sums the 8 partials and transposes. No collectives.
"""

import os
import sys

if "/opt/trn_rl_repo" not in sys.path:
    sys.path.insert(0, "/opt/trn_rl_repo")

import numpy as np
import ml_dtypes

from concourse import bacc, mybir, tile
from concourse import bass
from concourse.bass_utils import run_bass_kernel_spmd

F32 = mybir.dt.float32
F32R = mybir.dt.float32r
BF16 = mybir.dt.bfloat16
EXPF = mybir.ActivationFunctionType.Exp

N_CORES = 8
S = 2048
H = 4096
N_HEADS = 32
D = 128                      # head dim
HPC = N_HEADS // N_CORES     # heads per core = 4
HC = HPC * D                 # per-core hidden slice = 512
CH = 512                     # seq chunk width
NCH = S // CH                # 4 chunks
KT_TILES = H // 128          # 32 contraction tiles for projections
SJT = S // 128               # 16 seq j-tiles
ROPE_BASE = 10000.0
NEG = -1.0e9

BFNP = ml_dtypes.bfloat16

last_exec_time_ns = None


def _r(x):
    return np.ascontiguousarray(x, dtype=np.float32)


def _b(x):
    return np.ascontiguousarray(np.asarray(x, np.float32).astype(BFNP))


def _pmajor(a, kt):
    """[kt*128, C] -> [128, kt, C] partition-major layout."""
    R, C = a.shape
    return np.ascontiguousarray(
        np.asarray(a).reshape(kt, 128, C).transpose(1, 0, 2))


def _build(causal: bool):
    nc = bacc.Bacc("TRN2", target_bir_lowering=False, debug=False,
                   num_devices=N_CORES)
    htb = nc.dram_tensor("htb", [128, KT_TILES, S], BF16,
                         kind="ExternalInput")
    wqb = nc.dram_tensor("wqb", [128, KT_TILES, HC], BF16,
                         kind="ExternalInput")
    wkb = nc.dram_tensor("wkb", [128, KT_TILES, HC], BF16,
                         kind="ExternalInput")
    wvb = nc.dram_tensor("wvb", [KT_TILES, 128, HC], BF16,
                         kind="ExternalInput")
    wob = nc.dram_tensor("wob", [128, HPC, H], BF16, kind="ExternalInput")
    cosb = nc.dram_tensor("cosb", [D, S], BF16, kind="ExternalInput")
    sinb = nc.dram_tensor("sinb", [D, S], BF16, kind="ExternalInput")
    if causal:
        mband = nc.dram_tensor("mband", [128, 896], BF16,
                               kind="ExternalInput")
    else:
        maskT = nc.dram_tensor("maskT", [S, S], F32, kind="ExternalInput")
    po = nc.dram_tensor("po", [H, S], BF16, kind="ExternalOutput")

    def mm(out, lhsT, rhs, start, stop):
        nc.tensor.matmul(out, lhsT, rhs, start=start, stop=stop)

    from contextlib import ExitStack
    with tile.TileContext(nc) as tc:
        at_pool_cm = tc.tile_pool(name="at", bufs=1)
        at_pool = at_pool_cm.__enter__()
        AT = at_pool.tile([128, HPC, S], BF16, tag="at", name="AT")

        es_res = ExitStack()
        kt_pool = es_res.enter_context(tc.tile_pool(name="kt", bufs=HPC))
        v_pool = es_res.enter_context(tc.tile_pool(name="v", bufs=SJT))
        wqk_pool = es_res.enter_context(tc.tile_pool(name="wqk", bufs=1))
        KT = [kt_pool.tile([128, S], BF16, tag="kt", name=f"KT{i}")
              for i in range(HPC)]
        V = [v_pool.tile([128, HC], BF16, tag="v", name=f"V{i}")
             for i in range(SJT)]
        WQ = wqk_pool.tile([128, KT_TILES, HC], BF16, tag="wq", name="WQ")
        WK = wqk_pool.tile([128, KT_TILES, HC], BF16, tag="wk", name="WK")
        # split loads in groups of 4 k-tiles: trickle-feeds the Q pass
        for g in range(8):
            nc.sync.dma_start(out=WQ[:, bass.ts(g, 4), :],
                              in_=wqb[:, bass.ts(g, 4), :])
        for g in range(8):
            nc.scalar.dma_start(out=WK[:, bass.ts(g, 4), :],
                                in_=wkb[:, bass.ts(g, 4), :])

        with tc.tile_pool(name="qtc", bufs=4) as qtp, \
             tc.tile_pool(name="ht", bufs=1) as htp, \
             tc.tile_pool(name="wvs", bufs=6) as wvp, \
             tc.tile_pool(name="rope", bufs=2) as rp, \
             tc.tile_pool(name="aconst", bufs=1) as cpool, \
             tc.tile_pool(name="aes", bufs=2) as esp, \
             tc.tile_pool(name="am", bufs=1 if causal else 4) as mpool, \
             tc.tile_pool(name="ar", bufs=1) as rpool, \
             tc.tile_pool(name="mainps", bufs=2, space="PSUM") as psp:
            ones_col32 = cpool.tile([128, 1], F32, tag="oc32")
            nc.vector.memset(ones_col32[:], 1.0)
            ones_col = cpool.tile([128, 1], BF16, tag="oc")
            nc.vector.tensor_copy(ones_col[:], ones_col32[:])
            cosT = cpool.tile([128, S], BF16, tag="cos", name="cosT")
            sinT = cpool.tile([128, S], BF16, tag="sin", name="sinT")
            nc.scalar.dma_start(out=cosT[:], in_=cosb[:, :])
            nc.scalar.dma_start(out=sinT[:], in_=sinb[:, :])
            if causal:
                mb = cpool.tile([128, 896], BF16, tag="mb", name="mb")
                nc.scalar.dma_start(out=mb[:], in_=mband[:, :])

            def rope_evict(ps, dst_ap, c):
                # dst = psum*cos + shift(psum)*sin_signed
                cosc = cosT[:, bass.ts(c, CH)]
                sinc = sinT[:, bass.ts(c, CH)]
                raw = rp.tile([128, CH], BF16, tag="raw", name="raw")
                nc.scalar.copy(out=raw[:], in_=ps)
                shf = rp.tile([128, CH], BF16, tag="shf", name="shf")
                nc.gpsimd.dma_start(out=shf[0:64, :], in_=raw[64:128, :])
                nc.gpsimd.dma_start(out=shf[64:128, :], in_=raw[0:64, :])
                tmp = rp.tile([128, CH], BF16, tag="tmp", name="tmp")
                nc.vector.tensor_mul(tmp[:], shf[:], sinc)
                nc.vector.tensor_mul(dst_ap, raw[:], cosc)
                nc.vector.tensor_add(dst_ap, dst_ap, tmp[:])

            def load_ht(c):
                ht_t = htp.tile([128, KT_TILES, CH], BF16, tag="ht",
                                name="ht_t")
                for g in range(8):
                    nc.sync.dma_start(out=ht_t[:, bass.ts(g, 4), :],
                                      in_=htb[:, bass.ts(g, 4),
                                              bass.ts(c, CH)])
                return ht_t

            HT = load_ht(0)
            for c in range(NCH):
                # ---- Q pass (head-dim pairs, early evict) ----
                QTc = [qtp.tile([128, CH], BF16, tag="qtc", name=f"QTc{i}")
                       for i in range(HPC)]
                for dp in range(2):
                    qp2 = psp.tile([128, 2, CH], F32, tag="big",
                                   name="qp2")
                    for k in range(KT_TILES):
                        st, sp = (k == 0), (k == KT_TILES - 1)
                        for t in range(2):
                            d = 2 * dp + t
                            mm(qp2[:, t, :], WQ[:, k, bass.ts(d, 128)],
                               HT[:, k, :], st, sp)
                    for t in range(2):
                        rope_evict(qp2[:, t, :], QTc[2 * dp + t][:], c)
                # ---- K pass ----
                for dp in range(2):
                    kp2 = psp.tile([128, 2, CH], F32, tag="big",
                                   name="kp2")
                    for k in range(KT_TILES):
                        st, sp = (k == 0), (k == KT_TILES - 1)
                        for t in range(2):
                            d = 2 * dp + t
                            mm(kp2[:, t, :], WK[:, k, bass.ts(d, 128)],
                               HT[:, k, :], st, sp)
                    for t in range(2):
                        rope_evict(kp2[:, t, :],
                                   KT[2 * dp + t][:, bass.ts(c, CH)], c)
                # ---- V pass (wv streamed, both jl-pairs live) ----
                vp2 = [psp.tile([128, 2, CH], F32, tag="big",
                                name=f"vp2_{p}") for p in range(2)]
                for k in range(KT_TILES):
                    wv_t = wvp.tile([128, HC], BF16, tag="wv", name="wv_t")
                    nc.gpsimd.dma_start(out=wv_t[:], in_=wvb[k])
                    st, sp = (k == 0), (k == KT_TILES - 1)
                    for jl in range(4):
                        mm(vp2[jl // 2][:, jl % 2, :],
                           HT[:, k, bass.ts(jl, 128)], wv_t[:], st, sp)
                for jl in range(4):
                    nc.scalar.copy(out=V[4 * c + jl][:],
                                   in_=vp2[jl // 2][:, jl % 2, :])

                # prefetch next chunk's hidden tile during attention
                if c + 1 < NCH:
                    HT = load_ht(c + 1)

                # ---- attention for i-chunk c (K/V chunks <= c) ----
                ic = c
                jp_max = (2 * ic + 2) if causal else (SJT // 2)

                def finish_head(h_, sum_, o_):
                    rsum = rpool.tile([1, CH], F32, tag="rs", name="rsum")
                    rscr = rpool.tile([1, CH], F32, tag="rscr",
                                      name="rscr")
                    nc.vector.reciprocal_approx_accurate(
                        out=rsum[:], in_=sum_[:], scratch=rscr[:])
                    rb = rpool.tile([128, CH], F32, tag="rb", name="rb")
                    nc.gpsimd.partition_broadcast(rb[:], rsum[:])
                    nc.vector.tensor_mul(
                        AT[:, h_, bass.ts(ic, CH)], o_[:], rb[:])

                pend = []

                def drain_one():
                    h_, jp_, es2_, sum_, o_ = pend.pop(0)
                    last = False
                    for t in range(2):
                        j = 2 * jp_ + t
                        stq = (j == 0)
                        last = (j == 2 * jp_max - 1)
                        mm(sum_[:], ones_col[:], es2_[:, t, :], stq, last)
                        mm(o_[:], V[j][:, bass.ts(h_, 128)], es2_[:, t, :],
                           stq, last)
                    if last:
                        finish_head(h_, sum_, o_)

                for h in range(HPC):
                    sum_ps = psp.tile([1, CH], F32, tag="sum", bufs=2,
                                      name="sum_ps")
                    o_ps = psp.tile([128, CH], F32, tag="o", bufs=2,
                                    name="o_ps")
                    for jp in range(jp_max):
                        s2 = psp.tile([128, 2, CH], F32, tag="big",
                                      name="s2")
                        for t in range(2):
                            j = 2 * jp + t
                            mm(s2[:, t, :], KT[h][:, bass.ts(j, 128)],
                               QTc[h][:], True, True)
                            if causal:
                                if j >= 4 * ic:
                                    off = 384 - (j - 4 * ic) * 128
                                    nc.vector.tensor_add(
                                        s2[:, t, :], s2[:, t, :],
                                        mb[:, off:off + CH])
                            else:
                                mt = mpool.tile([128, CH], F32, tag="mt",
                                                name="mt")
                                nc.sync.dma_start(
                                    out=mt[:],
                                    in_=maskT[bass.ts(j, 128),
                                              bass.ts(ic, CH)])
                                nc.vector.tensor_add(s2[:, t, :],
                                                     s2[:, t, :], mt[:])
                        es2 = esp.tile([128, 2, CH], BF16, tag="es",
                                       name="es2")
                        nc.scalar.activation(es2[:], s2[:], EXPF)
                        pend.append((h, jp, es2, sum_ps, o_ps))
                        if len(pend) > 1:
                            drain_one()
                while pend:
                    drain_one()
        es_res.close()   # free weights/KT/V/ht SBUF before o_proj

        # ---------- o_proj  po = wo^T @ attnT ----------
        with tc.tile_pool(name="o_wo", bufs=1) as wop, \
             tc.tile_pool(name="o_out", bufs=8) as outp, \
             tc.tile_pool(name="o_ps", bufs=8, space="PSUM") as psp:
            WOa = wop.tile([128, HPC, H], BF16, tag="wo", name="WOa")
            for kl in range(HPC):
                nc.sync.dma_start(out=WOa[:, kl, :], in_=wob[:, kl, :])
            NB = 4           # n-tiles per block; kl-outer within a block
            for ic in range(NCH):
                for nb in range(H // 128 // NB):
                    pps = [psp.tile([128, CH], F32, tag="ps", name="pps")
                           for _ in range(NB)]
                    for kl in range(HPC):
                        for i in range(NB):
                            n = nb * NB + i
                            mm(pps[i][:], WOa[:, kl, bass.ts(n, 128)],
                               AT[:, kl, bass.ts(ic, CH)],
                               kl == 0, kl == HPC - 1)
                    for i in range(NB):
                        n = nb * NB + i
                        ot = outp.tile([128, CH], BF16, tag="ot", name="ot")
                        nc.scalar.copy(out=ot[:], in_=pps[i][:])
                        nc.gpsimd.dma_start(
                            out=po[bass.ts(n, 128), bass.ts(ic, CH)],
                            in_=ot[:])
        at_pool_cm.__exit__(None, None, None)
    nc.compile()
    return nc


_CACHE = {}


def _get_nc(causal):
    if causal not in _CACHE:
        _CACHE[causal] = _build(causal)
    return _CACHE[causal]


def kernel(hidden_states, attention_mask, position_ids, Wq, Wk, Wv, Wo):
    global last_exec_time_ns
    B, S_, H_ = hidden_states.shape
    assert (B, S_, H_) == (1, S, H)
    hs = np.asarray(hidden_states, dtype=np.float32)
    mask = np.asarray(attention_mask, dtype=np.float32)[0, 0]
    pos = np.asarray(position_ids)[0].astype(np.float64)

    # causal-mask fast path check
    iu = np.triu_indices(S, k=1)
    il = np.tril_indices(S, k=0)
    causal = bool(np.all(mask[il] == 0.0) and np.all(mask[iu] <= -1e30))

    hT = np.asarray(hs[0]).T               # [H, S]
    scale = 1.0 / np.sqrt(D)

    inv_freq = 1.0 / (ROPE_BASE ** (np.arange(0, D, 2, dtype=np.float64) / D))
    ang = pos[None, :] * np.concatenate([inv_freq, inv_freq])[:, None]  # [D,S]
    cosb = _b(np.cos(ang))
    sgn = np.ones((D, 1)); sgn[: D // 2] = -1.0
    sinb = _b(np.sin(ang) * sgn)

    htb = _pmajor(_b(hT), KT_TILES)
    wq_s = _b(np.asarray(Wq, np.float64) * scale)
    wk_b = _b(Wk)
    wv_b = _b(Wv)
    wo_b = _b(Wo)

    if causal:
        # band mask tile [128, 896]: mb[r, y] = NEG iff r > y - 384
        rr = np.arange(128)[:, None]
        yy = np.arange(896)[None, :]
        mband = np.ascontiguousarray(
            np.where(rr > yy - 384, NEG, 0.0).astype(BFNP))
    else:
        maskT = _r(mask.T)

    nc = _get_nc(causal)
    in_maps = []
    for c in range(N_CORES):
        sl = slice(c * HC, (c + 1) * HC)
        m = {
            "htb": htb,
            "wqb": _pmajor(wq_s[:, sl], KT_TILES),
            "wkb": _pmajor(wk_b[:, sl], KT_TILES),
            "wvb": np.ascontiguousarray(wv_b[:, sl]).reshape(
                KT_TILES, 128, HC),
            "wob": _pmajor(wo_b[sl, :], HPC),
            "cosb": cosb,
            "sinb": sinb,
        }
        if causal:
            m["mband"] = mband
        else:
            m["maskT"] = maskT
        in_maps.append(m)

    trace = bool(int(os.environ.get("BASS_KERNEL_TRACE", "0")))
    kw = {}
    if trace:
        kw["trace"] = True
        kw["tmpdir"] = os.environ.get("BASS_KERNEL_TRACE_DIR") or None
    res = run_bass_kernel_spmd(nc, in_maps, list(range(N_CORES)), **kw)
    last_exec_time_ns = res.exec_time_ns

    acc = np.zeros((H, S), dtype=np.float32)
    for c in range(N_CORES):
        acc += res.results[c]["po"].astype(np.float32)
    out = acc.T.reshape(1, S, H)
    return out


# revision 26
# speedup vs baseline: 1.4353x; 1.0218x over previous
"""Trainium2 Bass kernel for LlamaAttention (B=1, S=2048, H=4096, 32 heads).

Sharding: tensor-parallel over heads. 8 cores x 4 heads. Each core:
  - QKV projections in bf16 (1 cyc/out-col on the PE at 2.4 GHz; fp8
    DoubleRow measured at the same out-col rate, so bf16 wins once
    accuracy needs >1 fp8 pass). Wq/Wk resident in SBUF as single
    p-major tiles (few large DMAs; packets of one DMA spread over all
    16 DMA engines), Wv streamed per chunk.
  - head-dim PAIR accumulation in [128,2,CH] PSUM tiles (2 banks) with
    early eviction, so Q/K/V pass transitions don't stall; RoPE on
    Q^T/K^T (rotate-half = partition swap via SBUF DMA).
  - causal attention in transposed layout (keys on partitions), bf16
    scores / exp / PV; j-tile pairs share one [128,1024] exp;
    globally software-pipelined across heads so exp always overlaps
    matmuls; fully-masked blocks skipped; softmax without max
    subtraction; column sums via ones-matmul; 1/sum broadcast via
    gpsimd partition_broadcast (no PSUM bank).
  - attention output kept in SBUF (bf16); o_proj bf16, WO load
    overlapped into the last attention chunk; partial po written bf16.
  Host sums the 8 partials and transposes. No collectives.
"""

import os
import sys

if "/opt/trn_rl_repo" not in sys.path:
    sys.path.insert(0, "/opt/trn_rl_repo")

import numpy as np
import ml_dtypes

from concourse import bacc, mybir, tile
from concourse import bass
from concourse.bass_utils import run_bass_kernel_spmd

F32 = mybir.dt.float32
F32R = mybir.dt.float32r
BF16 = mybir.dt.bfloat16
EXPF = mybir.ActivationFunctionType.Exp

N_CORES = 8
S = 2048
H = 4096
N_HEADS = 32
D = 128                      # head dim
HPC = N_HEADS // N_CORES     # heads per core = 4
HC = HPC * D                 # per-core hidden slice = 512
CH = 512                     # seq chunk width
NCH = S // CH                # 4 chunks
KT_TILES = H // 128          # 32 contraction tiles for projections
SJT = S // 128               # 16 seq j-tiles
ROPE_BASE = 10000.0
NEG = -1.0e9

BFNP = ml_dtypes.bfloat16

last_exec_time_ns = None


def _r(x):
    return np.ascontiguousarray(x, dtype=np.float32)


def _b(x):
    return np.ascontiguousarray(np.asarray(x, np.float32).astype(BFNP))


def _pmajor(a, kt):
    """[kt*128, C] -> [128, kt, C] partition-major layout."""
    R, C = a.shape
    return np.ascontiguousarray(
        np.asarray(a).reshape(kt, 128, C).transpose(1, 0, 2))


def _build(causal: bool):
    nc = bacc.Bacc("TRN2", target_bir_lowering=False, debug=False,
                   num_devices=N_CORES)
    htb = nc.dram_tensor("htb", [128, KT_TILES, S], BF16,
                         kind="ExternalInput")
    wqb = nc.dram_tensor("wqb", [128, KT_TILES, HC], BF16,
                         kind="ExternalInput")
    wkb = nc.dram_tensor("wkb", [128, KT_TILES, HC], BF16,
                         kind="ExternalInput")
    wvb = nc.dram_tensor("wvb", [KT_TILES, 128, HC], BF16,
                         kind="ExternalInput")
    wob = nc.dram_tensor("wob", [128, HPC, H], BF16, kind="ExternalInput")
    cosb = nc.dram_tensor("cosb", [D, S], BF16, kind="ExternalInput")
    sinb = nc.dram_tensor("sinb", [D, S], BF16, kind="ExternalInput")
    if causal:
        mband = nc.dram_tensor("mband", [128, 896], BF16,
                               kind="ExternalInput")
    else:
        maskT = nc.dram_tensor("maskT", [S, S], F32, kind="ExternalInput")
    po = nc.dram_tensor("po", [H, S], BF16, kind="ExternalOutput")

    def mm(out, lhsT, rhs, start, stop):
        nc.tensor.matmul(out, lhsT, rhs, start=start, stop=stop)

    from contextlib import ExitStack
    with tile.TileContext(nc) as tc:
        at_pool_cm = tc.tile_pool(name="at", bufs=1)
        at_pool = at_pool_cm.__enter__()
        AT = at_pool.tile([128, HPC, S], BF16, tag="at", name="AT")

        es_res = ExitStack()
        kt_pool = es_res.enter_context(tc.tile_pool(name="kt", bufs=HPC))
        v_pool = es_res.enter_context(tc.tile_pool(name="v", bufs=SJT))
        wqk_pool = es_res.enter_context(tc.tile_pool(name="wqk", bufs=1))
        KT = [kt_pool.tile([128, S], BF16, tag="kt", name=f"KT{i}")
              for i in range(HPC)]
        V = [v_pool.tile([128, HC], BF16, tag="v", name=f"V{i}")
             for i in range(SJT)]
        WQ = wqk_pool.tile([128, KT_TILES, HC], BF16, tag="wq", name="WQ")
        WK = wqk_pool.tile([128, KT_TILES, HC], BF16, tag="wk", name="WK")

        with tc.tile_pool(name="qtc", bufs=4) as qtp, \
             tc.tile_pool(name="ht", bufs=1) as htp, \
             tc.tile_pool(name="wvs", bufs=6) as wvp, \
             tc.tile_pool(name="rope", bufs=2) as rp, \
             tc.tile_pool(name="aconst", bufs=1) as cpool, \
             tc.tile_pool(name="aes", bufs=2) as esp, \
             tc.tile_pool(name="am", bufs=1 if causal else 4) as mpool, \
             tc.tile_pool(name="ar", bufs=1) as rpool, \
             tc.tile_pool(name="mainps", bufs=2, space="PSUM") as psp:
            def load_ht(c):
                ht_t = htp.tile([128, KT_TILES, CH], BF16, tag="ht",
                                name="ht_t")
                for g in range(8):
                    nc.sync.dma_start(out=ht_t[:, bass.ts(g, 4), :],
                                      in_=htb[:, bass.ts(g, 4),
                                              bass.ts(c, CH)])
                return ht_t

            # preload: ht chunk 0 on sync, WQ on scalar, WK on gpsimd —
            # three dispatch queues run the transfers concurrently and
            # trickle-feed the first Q pass
            HT = load_ht(0)
            for g in range(8):
                nc.scalar.dma_start(out=WQ[:, bass.ts(g, 4), :],
                                    in_=wqb[:, bass.ts(g, 4), :])
            for g in range(8):
                nc.gpsimd.dma_start(out=WK[:, bass.ts(g, 4), :],
                                    in_=wkb[:, bass.ts(g, 4), :])
            ones_col32 = cpool.tile([128, 1], F32, tag="oc32")
            nc.vector.memset(ones_col32[:], 1.0)
            ones_col = cpool.tile([128, 1], BF16, tag="oc")
            nc.vector.tensor_copy(ones_col[:], ones_col32[:])
            cosT = cpool.tile([128, S], BF16, tag="cos", name="cosT")
            sinT = cpool.tile([128, S], BF16, tag="sin", name="sinT")
            nc.scalar.dma_start(out=cosT[:], in_=cosb[:, :])
            nc.scalar.dma_start(out=sinT[:], in_=sinb[:, :])
            if causal:
                mb = cpool.tile([128, 896], BF16, tag="mb", name="mb")
                nc.scalar.dma_start(out=mb[:], in_=mband[:, :])

            def rope_evict(ps, dst_ap, c):
                # dst = psum*cos + shift(psum)*sin_signed
                cosc = cosT[:, bass.ts(c, CH)]
                sinc = sinT[:, bass.ts(c, CH)]
                raw = rp.tile([128, CH], BF16, tag="raw", name="raw")
                nc.scalar.copy(out=raw[:], in_=ps)
                shf = rp.tile([128, CH], BF16, tag="shf", name="shf")
                nc.gpsimd.dma_start(out=shf[0:64, :], in_=raw[64:128, :])
                nc.gpsimd.dma_start(out=shf[64:128, :], in_=raw[0:64, :])
                tmp = rp.tile([128, CH], BF16, tag="tmp", name="tmp")
                nc.vector.tensor_mul(tmp[:], shf[:], sinc)
                nc.vector.tensor_mul(dst_ap, raw[:], cosc)
                nc.vector.tensor_add(dst_ap, dst_ap, tmp[:])

            for c in range(NCH):
                # ---- Q pass (head-dim pairs, early evict) ----
                QTc = [qtp.tile([128, CH], BF16, tag="qtc", name=f"QTc{i}")
                       for i in range(HPC)]
                for dp in range(2):
                    qp2 = psp.tile([128, 2, CH], F32, tag="big",
                                   name="qp2")
                    for k in range(KT_TILES):
                        st, sp = (k == 0), (k == KT_TILES - 1)
                        for t in range(2):
                            d = 2 * dp + t
                            mm(qp2[:, t, :], WQ[:, k, bass.ts(d, 128)],
                               HT[:, k, :], st, sp)
                    for t in range(2):
                        rope_evict(qp2[:, t, :], QTc[2 * dp + t][:], c)
                # ---- K pass ----
                for dp in range(2):
                    kp2 = psp.tile([128, 2, CH], F32, tag="big",
                                   name="kp2")
                    for k in range(KT_TILES):
                        st, sp = (k == 0), (k == KT_TILES - 1)
                        for t in range(2):
                            d = 2 * dp + t
                            mm(kp2[:, t, :], WK[:, k, bass.ts(d, 128)],
                               HT[:, k, :], st, sp)
                    for t in range(2):
                        rope_evict(kp2[:, t, :],
                                   KT[2 * dp + t][:, bass.ts(c, CH)], c)
                # ---- V pass (wv streamed, both jl-pairs live) ----
                vp2 = [psp.tile([128, 2, CH], F32, tag="big",
                                name=f"vp2_{p}") for p in range(2)]
                for k in range(KT_TILES):
                    wv_t = wvp.tile([128, HC], BF16, tag="wv", name="wv_t")
                    nc.gpsimd.dma_start(out=wv_t[:], in_=wvb[k])
                    st, sp = (k == 0), (k == KT_TILES - 1)
                    for jl in range(4):
                        mm(vp2[jl // 2][:, jl % 2, :],
                           HT[:, k, bass.ts(jl, 128)], wv_t[:], st, sp)
                for jl in range(4):
                    nc.scalar.copy(out=V[4 * c + jl][:],
                                   in_=vp2[jl // 2][:, jl % 2, :])

                # prefetch next chunk's hidden tile during attention
                if c + 1 < NCH:
                    HT = load_ht(c + 1)

                # ---- attention for i-chunk c (K/V chunks <= c) ----
                ic = c
                jp_max = (2 * ic + 2) if causal else (SJT // 2)

                def finish_head(h_, sum_, o_):
                    rsum = rpool.tile([1, CH], F32, tag="rs", name="rsum")
                    rscr = rpool.tile([1, CH], F32, tag="rscr",
                                      name="rscr")
                    nc.vector.reciprocal_approx_accurate(
                        out=rsum[:], in_=sum_[:], scratch=rscr[:])
                    rb = rpool.tile([128, CH], F32, tag="rb", name="rb")
                    nc.gpsimd.partition_broadcast(rb[:], rsum[:])
                    nc.vector.tensor_mul(
                        AT[:, h_, bass.ts(ic, CH)], o_[:], rb[:])

                pend = []

                def drain_one():
                    h_, jp_, q0s_, es2_, sum_, o_ = pend.pop(0)
                    last = False
                    for t in range(2):
                        j = 2 * jp_ + t
                        q0 = q0s_[t]
                        stq = (j == 0)
                        last = (j == 2 * jp_max - 1)
                        mm(sum_[:, q0:CH], ones_col[:],
                           es2_[:, t, q0:CH], stq, last)
                        mm(o_[:, q0:CH], V[j][:, bass.ts(h_, 128)],
                           es2_[:, t, q0:CH], stq, last)
                    if last:
                        finish_head(h_, sum_, o_)

                for h in range(HPC):
                    sum_ps = psp.tile([1, CH], F32, tag="sum", bufs=2,
                                      name="sum_ps")
                    o_ps = psp.tile([128, CH], F32, tag="o", bufs=2,
                                    name="o_ps")
                    for jp in range(jp_max):
                        s2 = psp.tile([128, 2, CH], F32, tag="big",
                                      name="s2")
                        # scores first (both halves), mask adds after —
                        # avoids a tile-granularity WAR stall on s2
                        q0s = []
                        for t in range(2):
                            j = 2 * jp + t
                            tl = j - 4 * ic
                            q0 = tl * 128 if (causal and tl > 0) else 0
                            q0s.append(q0)
                            mm(s2[:, t, q0:CH],
                               KT[h][:, bass.ts(j, 128)],
                               QTc[h][:, q0:CH], True, True)
                        for t in range(2):
                            j = 2 * jp + t
                            if causal:
                                if j >= 4 * ic:
                                    q0 = q0s[t]
                                    nc.vector.tensor_add(
                                        s2[:, t, q0:CH], s2[:, t, q0:CH],
                                        mb[:, 384:384 + CH - q0])
                            else:
                                mt = mpool.tile([128, CH], F32, tag="mt",
                                                name="mt")
                                nc.sync.dma_start(
                                    out=mt[:],
                                    in_=maskT[bass.ts(j, 128),
                                              bass.ts(ic, CH)])
                                nc.vector.tensor_add(s2[:, t, :],
                                                     s2[:, t, :], mt[:])
                        es2 = esp.tile([128, 2, CH], BF16, tag="es",
                                       name="es2")
                        nc.scalar.activation(es2[:], s2[:], EXPF)
                        pend.append((h, jp, q0s, es2, sum_ps, o_ps))
                        if len(pend) > 1:
                            drain_one()
                while pend:
                    drain_one()
        es_res.close()   # free weights/KT/V/ht SBUF before o_proj

        # ---------- o_proj  po = wo^T @ attnT ----------
        with tc.tile_pool(name="o_wo", bufs=1) as wop, \
             tc.tile_pool(name="o_out", bufs=8) as outp, \
             tc.tile_pool(name="o_ps", bufs=8, space="PSUM") as psp:
            WOa = wop.tile([128, HPC, H], BF16, tag="wo", name="WOa")
            # split by n-range: block nb only needs split nb — trickle
            for nb in range(8):
                nc.sync.dma_start(out=WOa[:, :, bass.ts(nb, 512)],
                                  in_=wob[:, :, bass.ts(nb, 512)])
            NB = 4           # n-tiles per block; kl-outer within a block
            for ic in range(NCH):
                for nb in range(H // 128 // NB):
                    pps = [psp.tile([128, CH], F32, tag="ps", name="pps")
                           for _ in range(NB)]
                    for kl in range(HPC):
                        for i in range(NB):
                            n = nb * NB + i
                            mm(pps[i][:], WOa[:, kl, bass.ts(n, 128)],
                               AT[:, kl, bass.ts(ic, CH)],
                               kl == 0, kl == HPC - 1)
                    for i in range(NB):
                        n = nb * NB + i
                        ot = outp.tile([128, CH], BF16, tag="ot", name="ot")
                        nc.scalar.copy(out=ot[:], in_=pps[i][:])
                        nc.gpsimd.dma_start(
                            out=po[bass.ts(n, 128), bass.ts(ic, CH)],
                            in_=ot[:])
        at_pool_cm.__exit__(None, None, None)
    nc.compile()
    return nc


_CACHE = {}


def _get_nc(causal):
    if causal not in _CACHE:
        _CACHE[causal] = _build(causal)
    return _CACHE[causal]


def kernel(hidden_states, attention_mask, position_ids, Wq, Wk, Wv, Wo):
    global last_exec_time_ns
    B, S_, H_ = hidden_states.shape
    assert (B, S_, H_) == (1, S, H)
    hs = np.asarray(hidden_states, dtype=np.float32)
    mask = np.asarray(attention_mask, dtype=np.float32)[0, 0]
    pos = np.asarray(position_ids)[0].astype(np.float64)

    # causal-mask fast path check
    iu = np.triu_indices(S, k=1)
    il = np.tril_indices(S, k=0)
    causal = bool(np.all(mask[il] == 0.0) and np.all(mask[iu] <= -1e30))

    hT = np.asarray(hs[0]).T               # [H, S]
    scale = 1.0 / np.sqrt(D)

    inv_freq = 1.0 / (ROPE_BASE ** (np.arange(0, D, 2, dtype=np.float64) / D))
    ang = pos[None, :] * np.concatenate([inv_freq, inv_freq])[:, None]  # [D,S]
    cosb = _b(np.cos(ang))
    sgn = np.ones((D, 1)); sgn[: D // 2] = -1.0
    sinb = _b(np.sin(ang) * sgn)

    htb = _pmajor(_b(hT), KT_TILES)
    wq_s = _b(np.asarray(Wq, np.float64) * scale)
    wk_b = _b(Wk)
    wv_b = _b(Wv)
    wo_b = _b(Wo)

    if causal:
        # band mask tile [128, 896]: mb[r, y] = NEG iff r > y - 384
        rr = np.arange(128)[:, None]
        yy = np.arange(896)[None, :]
        mband = np.ascontiguousarray(
            np.where(rr > yy - 384, NEG, 0.0).astype(BFNP))
    else:
        maskT = _r(mask.T)

    nc = _get_nc(causal)
    in_maps = []
    for c in range(N_CORES):
        sl = slice(c * HC, (c + 1) * HC)
        m = {
            "htb": htb,
            "wqb": _pmajor(wq_s[:, sl], KT_TILES),
            "wkb": _pmajor(wk_b[:, sl], KT_TILES),
            "wvb": np.ascontiguousarray(wv_b[:, sl]).reshape(
                KT_TILES, 128, HC),
            "wob": _pmajor(wo_b[sl, :], HPC),
            "cosb": cosb,
            "sinb": sinb,
        }
        if causal:
            m["mband"] = mband
        else:
            m["maskT"] = maskT
        in_maps.append(m)

    trace = bool(int(os.environ.get("BASS_KERNEL_TRACE", "0")))
    kw = {}
    if trace:
        kw["trace"] = True
        kw["tmpdir"] = os.environ.get("BASS_KERNEL_TRACE_DIR") or None
    res = run_bass_kernel_spmd(nc, in_maps, list(range(N_CORES)), **kw)
    last_exec_time_ns = res.exec_time_ns

    acc = np.zeros((H, S), dtype=np.float32)
    for c in range(N_CORES):
        acc += res.results[c]["po"].astype(np.float32)
    out = acc.T.reshape(1, S, H)
    return out
